# revision 1
# baseline (speedup 1.0000x reference)
"""MoE routing kernel for Trainium2 (8 NeuronCores, data-parallel over batch).

Problem: x[B=8,S=2048,D=1024] f32; gate Wg[E=4,D]+bg; experts We[E,D,D]+be.
  gate = x @ Wg.T + bg; top1 = argmax(gate); weights[b,e] = count_e(top1[b])/S
  out[b] = sum_e weights[b,e] * relu(x[b] @ We[e].T + be[e])

Sharding: batch dim across the 8 cores (1 batch element per core); expert
weights replicated. No collectives needed; host gathers per-core outputs.

Per-core kernel structure:
  - cast x and Wg to a bf16 hi/lo split on-chip; PE-transpose so the
    contraction dim (din) lands on partitions.
  - gate computed with (x_hi+x_lo)@(Wg_hi+Wg_lo).T accumulated in one PSUM
    tile (~fp32 accuracy, so argmax matches the f32 reference), then
    argmax->counts->weights entirely on-chip (is_ge + reductions + two tiny
    f32 matmuls for partition-sum and partition-broadcast).
  - expert matmuls in bf16 (PE 1 cyc/row vs 4 for f32), K=1024 contracted in
    8 chunks accumulating in PSUM, N=512 per matmul (one PSUM bank).
  - epilogue: relu(w_e * y) on ScalarE (w_e >= 0 so the weight folds into the
    activation scale, read from a per-partition SBUF scalar) + DVE add tree.
"""

import numpy as np

import concourse.bass as bass
import concourse.tile as tile
from concourse import mybir
from concourse.bass_utils import run_bass_kernel_spmd
from concourse.masks import make_identity
from concourse.vector_clock import ScopedClock, VectorClock

F32 = mybir.dt.float32
BF16 = mybir.dt.bfloat16
RELU = mybir.ActivationFunctionType.Relu
ALU = mybir.AluOpType

B, S, D, E = 8, 2048, 1024, 4
P = 128
NS = S // P   # 16 s-tiles
NK = D // P   # 8 contraction chunks
NC = 512      # matmul moving free dim (one PSUM bank of f32)
ND = D // NC  # 2 dout chunks


def _apply_tile_drain_patch():
    """The walrus build in this container only encodes one sync-wait on a
    CTRL instruction; Tile's kernel-tail drain attaches one wait per active
    proc to a single InstDrain and fails codegen. Split it into one drain
    per proc instead."""
    if getattr(tile.TileContext, "_moe_drain_patch", False):
        return
    tile.TileContext._moe_drain_patch = True

    def _drain_and_barrier(self, tick_clock, wait_clock):
        gc = tick_clock.global_clock
        scopes = [(None, gc)] if isinstance(gc, VectorClock) else gc.items()
        n_emitted = 0
        for scope, vc in scopes:
            n = len(vc)
            for proc in range(n):
                t = vc[proc]
                if t > 0:
                    single = VectorClock([t if i == proc else 0 for i in range(n)])
                    d = self.nc.sync.drain()
                    wait_clock.add_sem_waits(d.ins, ScopedClock({scope: single}))
                    n_emitted += 1
        if n_emitted == 0:
            self.nc.sync.drain()
        self.nc.all_engine_barrier()
        popped = self.nc._tile_sem_poison_stack.pop()
        assert popped is self._sem_poison
        self.nc.clear_and_free_semaphores(list(self.sems.allocated().values()))
        self.nc.all_engine_barrier()

    tile.TileContext._drain_and_barrier = _drain_and_barrier


_apply_tile_drain_patch()


def _split_sync_waits(nc: bass.Bass, limit: int = 1):
    """This container's walrus encodes at most one sync-wait per instruction.
    Hoist excess waits onto same-engine NoOps emitted immediately before the
    instruction — the engine stream blocks on each in turn, which is
    semantically identical to waiting on all of them at once."""
    ctr = 0
    for f in nc.m.functions:
        for bb in f.blocks:
            insts = list(bb.instructions)
            out = []
            changed = False
            for ins in insts:
                si = ins.sync_info
                waits = list(si.on_wait) if si is not None else []
                if len(waits) > limit:
                    changed = True
                    for w in waits[:-limit]:
                        ctr += 1
                        nop = mybir.InstNoOp(name=f"wsplit-{ctr}", ins=[], outs=[])
                        nop.engine = ins.engine
                        nop.sync_info = mybir.SyncInfo(on_wait=[w], on_update=[])
                        out.append(nop)
                    ins.sync_info = mybir.SyncInfo(
                        on_wait=waits[-limit:], on_update=list(si.on_update)
                    )
                out.append(ins)
            if changed:
                bb.instructions = out


def build_kernel(use_bg: bool, use_be: bool) -> bass.Bass:
    nc = bass.Bass()
    x_d = nc.dram_tensor("x", [S, D], F32, kind="ExternalInput")
    wg_d = nc.dram_tensor("Wg", [E, D], F32, kind="ExternalInput")
    bg_d = nc.dram_tensor("bg", [E], F32, kind="ExternalInput")
    we_d = nc.dram_tensor("We", [E, D, D], F32, kind="ExternalInput")
    be_d = nc.dram_tensor("be", [E, D], F32, kind="ExternalInput")
    out_d = nc.dram_tensor("out", [S, D], F32, kind="ExternalOutput")

    with tile.TileContext(nc) as tc:
        const = tc.alloc_tile_pool(name="const", bufs=1)
        big = tc.alloc_tile_pool(name="big", bufs=1)
        stage = tc.alloc_tile_pool(name="stage", bufs=4)
        stage_bf = tc.alloc_tile_pool(name="stage_bf", bufs=2)
        psum_tr = tc.alloc_tile_pool(name="psum_tr", bufs=3, space="PSUM")
        psum_gate = tc.alloc_tile_pool(name="psum_gate", bufs=2, space="PSUM")

        ident = const.tile([P, P], BF16)
        make_identity(nc, ident)
        ones_col_f = const.tile([P, 1], F32)
        nc.vector.memset(ones_col_f, 1.0)
        ones_row_f = const.tile([1, P], F32)
        nc.vector.memset(ones_row_f, 1.0)

        # --- gate weights: gather Wg transposed (din on partitions), split hi/lo
        # load Wg natural (one contiguous DMA), PE-transpose to [din, e]
        wg_sb = const.tile([E, D], F32)
        nc.sync.dma_start(out=wg_sb, in_=wg_d[:, :])
        ident_f = const.tile([P, P], F32)
        make_identity(nc, ident_f)
        pwg = psum_gate.tile([P, NK, E], F32, tag="pwg", bufs=1)
        for k in range(NK):
            nc.tensor.matmul(
                pwg[:, k, :],
                wg_sb[0:E, k * P : (k + 1) * P],
                ident_f[0:E, 0:E],
                is_transpose=True,
                start=True,
                stop=True,
            )
        wgT = const.tile([P, NK, E], F32)
        nc.scalar.copy(wgT, pwg)
        # rhs_cat[:, k, 0:4] = bf16(WgT), [:, k, 4:8] = WgT - hi
        rhs_cat = const.tile([P, NK, 2 * E], BF16)
        nc.vector.tensor_copy(rhs_cat[:, :, 0:E], wgT)
        nc.vector.tensor_sub(rhs_cat[:, :, E : 2 * E], wgT, rhs_cat[:, :, 0:E])

        if use_bg:
            bg_bc = const.tile([P, E], F32)
            nc.gpsimd.dma_start(
                out=bg_bc, in_=bass.AP(tensor=bg_d, offset=0, ap=[[0, P], [1, E]])
            )
        if use_be:
            be_f = const.tile([E, D], F32)
            nc.sync.dma_start(out=be_f, in_=be_d[:, :])
            be_bf = const.tile([E, D], BF16)
            nc.vector.tensor_copy(be_bf, be_f)
            ones_row_bf = const.tile([1, P], BF16)
            nc.vector.memset(ones_row_bf, 1.0)

        # --- persistent transposed operands
        xhT = big.tile([P, NK, NS, P], BF16)   # 32 KB/partition
        xlT = big.tile([P, NK, NS, P], BF16)   # 32 KB/partition
        weT = big.tile([P, E, NK, D], BF16)    # 64 KB/partition
        gate_all = const.tile([P, NS, E], F32)

        # --- x prep: load, hi/lo split, PE-transpose both ---
        for st in range(NS):
            x_nat = stage.tile([P, D], F32, tag="stg")
            nc.sync.dma_start(out=x_nat, in_=x_d[st * P : (st + 1) * P, :])
            x_hi = stage_bf.tile([P, D], BF16, tag="xhi")
            nc.vector.tensor_copy(x_hi, x_nat)
            x_lo = stage_bf.tile([P, D], BF16, tag="xlo")
            nc.vector.tensor_sub(x_lo, x_nat, x_hi)
            for src, dstT in ((x_hi, xhT), (x_lo, xlT)):
                ptr = psum_tr.tile([P, NK, P], BF16, tag="ptr")
                for k in range(NK):
                    nc.tensor.matmul(
                        ptr[:, k, :],
                        src[:, k * P : (k + 1) * P],
                        ident,
                        is_transpose=True,
                        start=True,
                        stop=True,
                    )
                nc.scalar.copy(dstT[:, :, st, :], ptr)

        # --- We prep: load, cast, PE-transpose ---
        for e in range(E):
            for dc in range(NK):  # 8 dout-chunks of 128 rows
                we_nat = stage.tile([P, D], F32, tag="stg")
                nc.sync.dma_start(
                    out=we_nat,
                    in_=we_d[e, dc * P : (dc + 1) * P, :],
                )
                we_bf = stage_bf.tile([P, D], BF16, tag="webf")
                nc.vector.tensor_copy(we_bf, we_nat)
                ptr = psum_tr.tile([P, NK, P], BF16, tag="ptr")
                for k in range(NK):
                    nc.tensor.matmul(
                        ptr[:, k, :],
                        we_bf[:, k * P : (k + 1) * P],
                        ident,
                        is_transpose=True,
                        start=True,
                        stop=True,
                    )
                nc.vector.tensor_copy(weT[:, e, :, dc * P : (dc + 1) * P], ptr)

        # --- gate matmuls: psum[:, 0, :] += x_hiT.T @ [Wg_hi|Wg_lo],
        #                   psum[:, 1, :] += x_loT.T @ [Wg_hi|Wg_lo]
        for st in range(NS):
            # two PSUM banks: interleaved accumulation groups must not share a
            # bank (start=True clears has_written for the whole bank)
            pg = psum_gate.tile([P, 2, NC], F32, tag="pg", bufs=1)
            for k in range(NK):
                nc.tensor.matmul(
                    pg[:, 0, 0 : 2 * E], xhT[:, k, st, :], rhs_cat[:, k, :],
                    start=(k == 0), stop=(k == NK - 1),
                )
                nc.tensor.matmul(
                    pg[:, 1, 0 : 2 * E], xlT[:, k, st, :], rhs_cat[:, k, :],
                    start=(k == 0), stop=(k == NK - 1),
                )
            # gate[s, e] = sum over the 4 groups {x_hi,x_lo}x{Wg_hi,Wg_lo}
            gview = bass.AP(
                tensor=pg.tensor, offset=pg.offset,
                ap=[pg.ap[0], [1, E], [NC, 2], [E, 2]],
            )
            if use_bg:
                gtmp = stage.tile([P, E], F32, tag="gtmp")
                nc.vector.tensor_reduce(
                    gtmp, gview, axis=mybir.AxisListType.XY, op=ALU.add
                )
                nc.vector.tensor_add(gate_all[:, st, :], gtmp, bg_bc)
            else:
                nc.vector.tensor_reduce(
                    gate_all[:, st, :], gview, axis=mybir.AxisListType.XY, op=ALU.add
                )

        # --- counts -> weights (broadcast to all partitions) ---
        rowmax = const.tile([P, NS], F32)
        nc.vector.tensor_reduce(rowmax, gate_all, axis=mybir.AxisListType.X, op=ALU.max)
        ismax = const.tile([P, E, NS], F32)
        g_ens = gate_all.rearrange("p n e -> p e n")
        rm_bc = bass.AP(
            tensor=rowmax.tensor, offset=rowmax.offset,
            ap=[rowmax.ap[0], [0, E], [1, NS]],
        )
        nc.vector.tensor_tensor(ismax, g_ens, rm_bc, op=ALU.is_ge)
        counts_part = const.tile([P, E], F32)
        nc.vector.tensor_reduce(
            counts_part, ismax, axis=mybir.AxisListType.X, op=ALU.add
        )

        pc1 = psum_gate.tile([1, E], F32, tag="pc1", bufs=1)
        nc.tensor.matmul(pc1, ones_col_f, counts_part, start=True, stop=True)
        counts_sb = const.tile([1, E], F32)
        nc.scalar.copy(counts_sb, pc1)
        pc2 = psum_gate.tile([P, E], F32, tag="pc2", bufs=1)
        nc.tensor.matmul(pc2, ones_row_f, counts_sb, start=True, stop=True)
        w_bc = const.tile([P, E], F32)
        nc.scalar.mul(w_bc, pc2, 1.0 / S)

        psum_gate.release()
        psum_tr.release()
        stage_bf.release()

        # --- main expert matmuls + fused epilogue ---
        psum_main = tc.alloc_tile_pool(name="psum_main", bufs=4, space="PSUM")
        relu_p = tc.alloc_tile_pool(name="relu_p", bufs=6)
        acc_p = tc.alloc_tile_pool(name="acc_p", bufs=4)
        out_p = tc.alloc_tile_pool(name="out_p", bufs=3)

        for st in range(NS):
            accs = []
            for half in range(2):
                pts = [
                    psum_main.tile([P, D], F32, tag="pm", name=f"pm{e2}")
                    for e2 in range(2)
                ]
                if use_be:
                    for e2, pt in enumerate(pts):
                        e = half * 2 + e2
                        for c in range(ND):
                            nc.tensor.matmul(
                                pt[:, c * NC : (c + 1) * NC],
                                ones_row_bf,
                                be_bf[e : e + 1, c * NC : (c + 1) * NC],
                                start=True, stop=False,
                            )
                for k in range(NK):
                    lhs = xhT[:, k, st, :]
                    for e2, pt in enumerate(pts):
                        for c in range(ND):
                            e = half * 2 + e2
                            nc.tensor.matmul(
                                pt[:, c * NC : (c + 1) * NC],
                                lhs,
                                weT[:, e, k, c * NC : (c + 1) * NC],
                                start=(k == 0 and not use_be),
                                stop=(k == NK - 1),
                            )
                trs = []
                for e2, pt in enumerate(pts):
                    e = half * 2 + e2
                    tr = relu_p.tile([P, D], BF16, tag="tr")
                    nc.scalar.activation(tr, pt, RELU, scale=w_bc[:, e : e + 1])
                    trs.append(tr)
                acc = acc_p.tile([P, D], F32, tag="acc")
                nc.vector.tensor_add(acc, trs[0], trs[1])
                accs.append(acc)
            o = out_p.tile([P, D], F32, tag="o")
            nc.vector.tensor_add(o, accs[0], accs[1])
            nc.sync.dma_start(out=out_d[st * P : (st + 1) * P, :], in_=o)

        out_p.release()
        acc_p.release()
        relu_p.release()
        psum_main.release()
        stage.release()
        big.release()
        const.release()

    _split_sync_waits(nc)
    return nc


_CACHE = {}


def _get_kernel(use_bg: bool, use_be: bool) -> bass.Bass:
    key = (use_bg, use_be)
    if key not in _CACHE:
        _CACHE[key] = build_kernel(use_bg, use_be)
    return _CACHE[key]


def kernel(x, Wg, bg, We, be, _trace=False):
    x = np.ascontiguousarray(np.asarray(x, dtype=np.float32))
    Wg = np.ascontiguousarray(np.asarray(Wg, dtype=np.float32))
    bg = np.ascontiguousarray(np.asarray(bg, dtype=np.float32))
    We = np.ascontiguousarray(np.asarray(We, dtype=np.float32))
    be = np.ascontiguousarray(np.asarray(be, dtype=np.float32))
    assert x.shape == (B, S, D) and Wg.shape == (E, D)
    assert We.shape == (E, D, D) and bg.shape == (E,) and be.shape == (E, D)

    use_bg = bool(np.any(bg))
    use_be = bool(np.any(be))
    nc = _get_kernel(use_bg, use_be)

    in_maps = [
        {"x": x[b], "Wg": Wg, "We": We, "bg": bg, "be": be} for b in range(B)
    ]
    try:
        res = run_bass_kernel_spmd(
            nc, in_maps, core_ids=list(range(B)), trace=_trace
        )
    except ModuleNotFoundError:
        # NTFF profile hook unavailable in this container; run untraced
        res = run_bass_kernel_spmd(nc, in_maps, core_ids=list(range(B)))
    out = np.stack([res.results[b]["out"] for b in range(B)], axis=0)
    if _trace:
        return out, res
    return out



# revision 2
# speedup vs baseline: 1.9231x; 1.9231x over previous
"""MoE routing kernel for Trainium2 (8 NeuronCores, data-parallel over batch).

Problem: x[B=8,S=2048,D=1024] f32; gate Wg[E=4,D]+bg; experts We[E,D,D]+be.
  gate = x @ Wg.T + bg; top1 = argmax(gate); weights[b,e] = count_e(top1[b])/S
  out[b] = sum_e weights[b,e] * relu(x[b] @ We[e].T + be[e])

The warm-path cost in this environment is dominated by host<->device traffic
over the axon tunnel (~38 MB/s), not device compute (~0.3 ms). So the design
minimizes bytes moved:
  - everything is cast to bf16 on the host (x, We, Wg, bg, be) and packed into
    ONE [6153, 1024] bf16 dram tensor per core (one transfer, low latency);
  - the device output is bf16 [S, D], upcast to f32 on the host;
  - measured end-to-end rel err of the full-bf16 pipeline: 5.3e-3 (tolerance
    2e-2; routing argmax from bf16 inputs flips only ~15/16384 tokens).

Sharding: batch dim across the 8 cores (1 batch element per core); expert
weights replicated. No collectives; host gathers per-core outputs.

Per-core kernel structure:
  - PE-transpose x and We tiles so the contraction dim (din) lands on
    partitions; all matmuls in bf16 (PE 1 cyc/row), f32 PSUM accumulation.
  - gate = xT.T @ WgT accumulated over 8 k-chunks in one PSUM tile, then
    argmax->counts->weights entirely on-chip (is_ge + reductions + two tiny
    f32 matmuls for partition-sum and partition-broadcast).
  - expert matmuls: K=1024 contracted in 8 chunks accumulating in PSUM,
    N=512 per matmul (one PSUM bank).
  - epilogue: relu(w_e * y) on ScalarE (w_e >= 0 so the weight folds into the
    activation scale) + DVE add tree, final add emits bf16 out tile.
"""

import numpy as np
import ml_dtypes

import concourse.bass as bass
import concourse.tile as tile
from concourse import mybir
from concourse.bass_utils import run_bass_kernel_spmd
from concourse.masks import make_identity
from concourse.vector_clock import ScopedClock, VectorClock

F32 = mybir.dt.float32
BF16 = mybir.dt.bfloat16
RELU = mybir.ActivationFunctionType.Relu
ALU = mybir.AluOpType
NPBF16 = ml_dtypes.bfloat16

B, S, D, E = 8, 2048, 1024, 4
P = 128
NS = S // P   # 16 s-tiles
NK = D // P   # 8 contraction chunks
NC = 512      # matmul moving free dim (one PSUM bank of f32)
ND = D // NC  # 2 dout chunks

# pack row layout (rows of D bf16 values)
R_X = 0                # x[b]:  S rows
R_WE = R_X + S         # We:    E*D rows (natural [dout, din] per expert)
R_WG = R_WE + E * D    # Wg:    E rows
R_BG = R_WG + E        # bg:    1 row (first E entries)
R_BE = R_BG + 1        # be:    E rows
R_TOT = R_BE + E       # 6153


def _apply_tile_drain_patch():
    """The walrus build in this container only encodes one sync-wait on a
    CTRL instruction; Tile's kernel-tail drain attaches one wait per active
    proc to a single InstDrain and fails codegen. Split it into one drain
    per proc instead."""
    if getattr(tile.TileContext, "_moe_drain_patch", False):
        return
    tile.TileContext._moe_drain_patch = True

    def _drain_and_barrier(self, tick_clock, wait_clock):
        gc = tick_clock.global_clock
        scopes = [(None, gc)] if isinstance(gc, VectorClock) else gc.items()
        n_emitted = 0
        for scope, vc in scopes:
            n = len(vc)
            for proc in range(n):
                t = vc[proc]
                if t > 0:
                    single = VectorClock([t if i == proc else 0 for i in range(n)])
                    d = self.nc.sync.drain()
                    wait_clock.add_sem_waits(d.ins, ScopedClock({scope: single}))
                    n_emitted += 1
        if n_emitted == 0:
            self.nc.sync.drain()
        self.nc.all_engine_barrier()
        popped = self.nc._tile_sem_poison_stack.pop()
        assert popped is self._sem_poison
        self.nc.clear_and_free_semaphores(list(self.sems.allocated().values()))
        self.nc.all_engine_barrier()

    tile.TileContext._drain_and_barrier = _drain_and_barrier


_apply_tile_drain_patch()


def _split_sync_waits(nc: bass.Bass, limit: int = 1):
    """This container's walrus encodes at most one sync-wait per instruction.
    Hoist excess waits onto same-engine NoOps emitted immediately before the
    instruction — the engine stream blocks on each in turn, which is
    semantically identical to waiting on all of them at once."""
    ctr = 0
    for f in nc.m.functions:
        for bb in f.blocks:
            insts = list(bb.instructions)
            out = []
            changed = False
            for ins in insts:
                si = ins.sync_info
                waits = list(si.on_wait) if si is not None else []
                if len(waits) > limit:
                    changed = True
                    for w in waits[:-limit]:
                        ctr += 1
                        nop = mybir.InstNoOp(name=f"wsplit-{ctr}", ins=[], outs=[])
                        nop.engine = ins.engine
                        nop.sync_info = mybir.SyncInfo(on_wait=[w], on_update=[])
                        out.append(nop)
                    ins.sync_info = mybir.SyncInfo(
                        on_wait=waits[-limit:], on_update=list(si.on_update)
                    )
                out.append(ins)
            if changed:
                bb.instructions = out


def build_kernel(use_bg: bool, use_be: bool) -> bass.Bass:
    nc = bass.Bass()
    pack_d = nc.dram_tensor("pack", [R_TOT, D], BF16, kind="ExternalInput")
    out_d = nc.dram_tensor("out", [S, D], BF16, kind="ExternalOutput")

    with tile.TileContext(nc) as tc:
        const = tc.alloc_tile_pool(name="const", bufs=1)
        big = tc.alloc_tile_pool(name="big", bufs=1)
        stage = tc.alloc_tile_pool(name="stage", bufs=4)
        psum_tr = tc.alloc_tile_pool(name="psum_tr", bufs=3, space="PSUM")
        psum_gate = tc.alloc_tile_pool(name="psum_gate", bufs=2, space="PSUM")

        ident = const.tile([P, P], BF16)
        make_identity(nc, ident)
        ones_col_f = const.tile([P, 1], F32)
        nc.vector.memset(ones_col_f, 1.0)
        ones_row_f = const.tile([1, P], F32)
        nc.vector.memset(ones_row_f, 1.0)

        # --- gate weights: load Wg rows, PE-transpose to [din, e] ---
        wg_sb = const.tile([E, D], BF16)
        nc.sync.dma_start(out=wg_sb, in_=pack_d[R_WG : R_WG + E, :])
        pwg = psum_gate.tile([P, NK, E], BF16, tag="pwg", bufs=1)
        for k in range(NK):
            nc.tensor.matmul(
                pwg[:, k, :],
                wg_sb[0:E, k * P : (k + 1) * P],
                ident[0:E, 0:E],
                is_transpose=True,
                start=True,
                stop=True,
            )
        wgT = const.tile([P, NK, E], BF16)
        nc.scalar.copy(wgT, pwg)

        if use_bg:
            bg_bc = const.tile([P, E], BF16)
            nc.gpsimd.dma_start(
                out=bg_bc,
                in_=bass.AP(tensor=pack_d, offset=R_BG * D, ap=[[0, P], [1, E]]),
            )
        if use_be:
            be_bf = const.tile([E, D], BF16)
            nc.sync.dma_start(out=be_bf, in_=pack_d[R_BE : R_BE + E, :])
            ones_row_bf = const.tile([1, P], BF16)
            nc.vector.memset(ones_row_bf, 1.0)

        # --- persistent transposed operands ---
        xT = big.tile([P, NK, NS, P], BF16)    # 32 KB/partition
        weT = big.tile([P, E, NK, D], BF16)    # 64 KB/partition
        gate_all = const.tile([P, NS, E], F32)

        # --- x prep: load bf16, PE-transpose ---
        for st in range(NS):
            x_nat = stage.tile([P, D], BF16, tag="stg")
            nc.sync.dma_start(out=x_nat, in_=pack_d[st * P : (st + 1) * P, :])
            ptr = psum_tr.tile([P, NK, P], BF16, tag="ptr")
            for k in range(NK):
                nc.tensor.matmul(
                    ptr[:, k, :],
                    x_nat[:, k * P : (k + 1) * P],
                    ident,
                    is_transpose=True,
                    start=True,
                    stop=True,
                )
            nc.scalar.copy(xT[:, :, st, :], ptr)

        # --- We prep: load bf16, PE-transpose ---
        for e in range(E):
            for dc in range(NK):  # 8 dout-chunks of 128 rows
                r0 = R_WE + e * D + dc * P
                we_nat = stage.tile([P, D], BF16, tag="stg")
                nc.sync.dma_start(out=we_nat, in_=pack_d[r0 : r0 + P, :])
                ptr = psum_tr.tile([P, NK, P], BF16, tag="ptr")
                for k in range(NK):
                    nc.tensor.matmul(
                        ptr[:, k, :],
                        we_nat[:, k * P : (k + 1) * P],
                        ident,
                        is_transpose=True,
                        start=True,
                        stop=True,
                    )
                nc.vector.tensor_copy(weT[:, e, :, dc * P : (dc + 1) * P], ptr)

        # --- gate matmuls: psum += xT.T @ WgT over 8 k-chunks ---
        for st in range(NS):
            pg = psum_gate.tile([P, E], F32, tag="pg")
            for k in range(NK):
                nc.tensor.matmul(
                    pg, xT[:, k, st, :], wgT[:, k, :],
                    start=(k == 0), stop=(k == NK - 1),
                )
            if use_bg:
                nc.vector.tensor_add(gate_all[:, st, :], pg, bg_bc)
            else:
                nc.scalar.copy(gate_all[:, st, :], pg)

        # --- counts -> weights (broadcast to all partitions) ---
        rowmax = const.tile([P, NS], F32)
        nc.vector.tensor_reduce(rowmax, gate_all, axis=mybir.AxisListType.X, op=ALU.max)
        ismax = const.tile([P, E, NS], F32)
        g_ens = gate_all.rearrange("p n e -> p e n")
        rm_bc = bass.AP(
            tensor=rowmax.tensor, offset=rowmax.offset,
            ap=[rowmax.ap[0], [0, E], [1, NS]],
        )
        nc.vector.tensor_tensor(ismax, g_ens, rm_bc, op=ALU.is_ge)
        counts_part = const.tile([P, E], F32)
        nc.vector.tensor_reduce(
            counts_part, ismax, axis=mybir.AxisListType.X, op=ALU.add
        )

        pc1 = psum_gate.tile([1, E], F32, tag="pc1", bufs=1)
        nc.tensor.matmul(pc1, ones_col_f, counts_part, start=True, stop=True)
        counts_sb = const.tile([1, E], F32)
        nc.scalar.copy(counts_sb, pc1)
        pc2 = psum_gate.tile([P, E], F32, tag="pc2", bufs=1)
        nc.tensor.matmul(pc2, ones_row_f, counts_sb, start=True, stop=True)
        w_bc = const.tile([P, E], F32)
        nc.scalar.mul(w_bc, pc2, 1.0 / S)

        psum_gate.release()
        psum_tr.release()

        # --- main expert matmuls + fused epilogue ---
        psum_main = tc.alloc_tile_pool(name="psum_main", bufs=4, space="PSUM")
        relu_p = tc.alloc_tile_pool(name="relu_p", bufs=6)
        acc_p = tc.alloc_tile_pool(name="acc_p", bufs=4)
        out_p = tc.alloc_tile_pool(name="out_p", bufs=3)

        for st in range(NS):
            accs = []
            for half in range(2):
                pts = [
                    psum_main.tile([P, D], F32, tag="pm", name=f"pm{e2}")
                    for e2 in range(2)
                ]
                if use_be:
                    for e2, pt in enumerate(pts):
                        e = half * 2 + e2
                        for c in range(ND):
                            nc.tensor.matmul(
                                pt[:, c * NC : (c + 1) * NC],
                                ones_row_bf,
                                be_bf[e : e + 1, c * NC : (c + 1) * NC],
                                start=True, stop=False,
                            )
                for k in range(NK):
                    lhs = xT[:, k, st, :]
                    for e2, pt in enumerate(pts):
                        for c in range(ND):
                            e = half * 2 + e2
                            nc.tensor.matmul(
                                pt[:, c * NC : (c + 1) * NC],
                                lhs,
                                weT[:, e, k, c * NC : (c + 1) * NC],
                                start=(k == 0 and not use_be),
                                stop=(k == NK - 1),
                            )
                trs = []
                for e2, pt in enumerate(pts):
                    e = half * 2 + e2
                    tr = relu_p.tile([P, D], BF16, tag="tr")
                    nc.scalar.activation(tr, pt, RELU, scale=w_bc[:, e : e + 1])
                    trs.append(tr)
                acc = acc_p.tile([P, D], F32, tag="acc")
                nc.vector.tensor_add(acc, trs[0], trs[1])
                accs.append(acc)
            o = out_p.tile([P, D], BF16, tag="o")
            nc.vector.tensor_add(o, accs[0], accs[1])
            nc.sync.dma_start(out=out_d[st * P : (st + 1) * P, :], in_=o)

        out_p.release()
        acc_p.release()
        relu_p.release()
        psum_main.release()
        stage.release()
        big.release()
        const.release()

    _split_sync_waits(nc)
    return nc


_CACHE = {}


def _get_kernel(use_bg: bool, use_be: bool) -> bass.Bass:
    key = (use_bg, use_be)
    if key not in _CACHE:
        _CACHE[key] = build_kernel(use_bg, use_be)
    return _CACHE[key]


def kernel(x, Wg, bg, We, be, _trace=False):
    x = np.asarray(x, dtype=np.float32)
    Wg = np.asarray(Wg, dtype=np.float32)
    bg = np.asarray(bg, dtype=np.float32)
    We = np.asarray(We, dtype=np.float32)
    be = np.asarray(be, dtype=np.float32)
    assert x.shape == (B, S, D) and Wg.shape == (E, D)
    assert We.shape == (E, D, D) and bg.shape == (E,) and be.shape == (E, D)

    use_bg = bool(np.any(bg))
    use_be = bool(np.any(be))
    nc = _get_kernel(use_bg, use_be)

    # host-side bf16 cast + pack: one input tensor per core
    x_bf = x.astype(NPBF16)
    tail = np.empty((R_TOT - S, D), dtype=NPBF16)
    tail[0 : E * D] = We.reshape(E * D, D).astype(NPBF16)
    tail[R_WG - S : R_WG - S + E] = Wg.astype(NPBF16)
    tail[R_BG - S] = 0
    tail[R_BG - S, 0:E] = bg.astype(NPBF16)
    tail[R_BE - S : R_BE - S + E] = be.astype(NPBF16)

    in_maps = [
        {"pack": np.concatenate([x_bf[b], tail], axis=0)} for b in range(B)
    ]
    try:
        res = run_bass_kernel_spmd(
            nc, in_maps, core_ids=list(range(B)), trace=_trace
        )
    except ModuleNotFoundError:
        # NTFF profile hook unavailable in this container; run untraced
        res = run_bass_kernel_spmd(nc, in_maps, core_ids=list(range(B)))
    out = np.stack([res.results[b]["out"] for b in range(B)], axis=0).astype(
        np.float32
    )
    if _trace:
        return out, res
    return out


# revision 8
# speedup vs baseline: 2.8749x; 1.4949x over previous
"""MoE routing kernel for Trainium2 (8 NeuronCores, data-parallel over batch).

Problem: x[B=8,S=2048,D=1024] f32; gate Wg[E=4,D]+bg; experts We[E,D,D]+be.
  gate = x @ Wg.T + bg; top1 = argmax(gate); weights[b,e] = count_e(top1[b])/S
  out[b] = sum_e weights[b,e] * relu(x[b] @ We[e].T + be[e])

The warm-path cost in this environment is dominated by host<->device traffic
over the axon tunnel (~38 MB/s), not device compute (~0.3 ms). So the design
minimizes bytes moved:
  - everything is cast to bf16 on the host (x, We, Wg, bg, be) and packed into
    ONE [6153, 1024] bf16 dram tensor per core (one transfer, low latency);
  - the device output is bf16 [S, D], upcast to f32 on the host;
  - measured end-to-end rel err of the full-bf16 pipeline: 5.3e-3 (tolerance
    2e-2; routing argmax from bf16 inputs flips only ~15/16384 tokens).

Sharding: batch dim across the 8 cores (1 batch element per core); expert
weights replicated. No collectives; host gathers per-core outputs.

Per-core kernel structure:
  - PE-transpose x and We tiles so the contraction dim (din) lands on
    partitions; all matmuls in bf16 (PE 1 cyc/row), f32 PSUM accumulation.
  - gate = xT.T @ WgT accumulated over 8 k-chunks in one PSUM tile, then
    argmax->counts->weights entirely on-chip (is_ge + reductions + two tiny
    f32 matmuls for partition-sum and partition-broadcast).
  - expert matmuls: K=1024 contracted in 8 chunks accumulating in PSUM,
    N=512 per matmul (one PSUM bank).
  - epilogue: relu(w_e * y) on ScalarE (w_e >= 0 so the weight folds into the
    activation scale) + DVE add tree, final add emits bf16 out tile.
"""

import numpy as np
import ml_dtypes

import jax

# Persist XLA executables across processes/calls: run_bass_via_pjrt re-jits a
# fresh closure every call, so without this each warm call pays ~0.5s of
# XLA+BIR recompile. Harmless no-op if the PJRT client can't serialize.
jax.config.update("jax_compilation_cache_dir", "/tmp/jax_comp_cache")
jax.config.update("jax_persistent_cache_min_compile_time_secs", 0.0)
jax.config.update("jax_persistent_cache_min_entry_size_bytes", 0)

import concourse.bass as bass
import concourse.tile as tile
from concourse import mybir
from concourse.bass_utils import run_bass_kernel_spmd
from concourse.masks import make_identity
from concourse.vector_clock import ScopedClock, VectorClock

F32 = mybir.dt.float32
BF16 = mybir.dt.bfloat16
RELU = mybir.ActivationFunctionType.Relu
ALU = mybir.AluOpType
NPBF16 = ml_dtypes.bfloat16

B, S, D, E = 8, 2048, 1024, 4
P = 128
NS = S // P   # 16 s-tiles
NK = D // P   # 8 contraction chunks
NC = 512      # matmul moving free dim (one PSUM bank of f32)
ND = D // NC  # 2 dout chunks

# pack row layout (rows of D bf16 values). We is sharded 1/8th per core and
# AllGathered on device — 8MB crosses the slow host tunnel once, not 8 times.
WE_SHARD = E * D // B  # 512 rows per core
R_X = 0                # x[b]:  S rows
R_WE = R_X + S         # We shard: rows [c*512, (c+1)*512) of We.reshape(E*D, D)
R_WG = R_WE + WE_SHARD # Wg:    E rows
R_BG = R_WG + E        # bg:    1 row (first E entries)
R_BE = R_BG + 1        # be:    E rows
R_TOT = R_BE + E       # 2569


def _apply_tile_drain_patch():
    """The walrus build in this container only encodes one sync-wait on a
    CTRL instruction; Tile's kernel-tail drain attaches one wait per active
    proc to a single InstDrain and fails codegen. Split it into one drain
    per proc instead."""
    if getattr(tile.TileContext, "_moe_drain_patch", False):
        return
    tile.TileContext._moe_drain_patch = True

    def _drain_and_barrier(self, tick_clock, wait_clock):
        gc = tick_clock.global_clock
        scopes = [(None, gc)] if isinstance(gc, VectorClock) else gc.items()
        n_emitted = 0
        for scope, vc in scopes:
            n = len(vc)
            for proc in range(n):
                t = vc[proc]
                if t > 0:
                    single = VectorClock([t if i == proc else 0 for i in range(n)])
                    d = self.nc.sync.drain()
                    wait_clock.add_sem_waits(d.ins, ScopedClock({scope: single}))
                    n_emitted += 1
        if n_emitted == 0:
            self.nc.sync.drain()
        self.nc.all_engine_barrier()
        popped = self.nc._tile_sem_poison_stack.pop()
        assert popped is self._sem_poison
        self.nc.clear_and_free_semaphores(list(self.sems.allocated().values()))
        self.nc.all_engine_barrier()

    tile.TileContext._drain_and_barrier = _drain_and_barrier


_apply_tile_drain_patch()


def _split_sync_waits(nc: bass.Bass, limit: int = 1):
    """This container's walrus encodes at most one sync-wait per instruction.
    Hoist excess waits onto same-engine NoOps emitted immediately before the
    instruction — the engine stream blocks on each in turn, which is
    semantically identical to waiting on all of them at once."""
    ctr = 0
    for f in nc.m.functions:
        for bb in f.blocks:
            insts = list(bb.instructions)
            out = []
            changed = False
            for ins in insts:
                si = ins.sync_info
                waits = list(si.on_wait) if si is not None else []
                if len(waits) > limit:
                    changed = True
                    for w in waits[:-limit]:
                        ctr += 1
                        nop = mybir.InstNoOp(name=f"wsplit-{ctr}", ins=[], outs=[])
                        nop.engine = ins.engine
                        nop.sync_info = mybir.SyncInfo(on_wait=[w], on_update=[])
                        out.append(nop)
                    ins.sync_info = mybir.SyncInfo(
                        on_wait=waits[-limit:], on_update=list(si.on_update)
                    )
                out.append(ins)
            if changed:
                bb.instructions = out


def build_kernel(use_bg: bool, use_be: bool) -> bass.Bass:
    nc = bass.Bass()
    pack_d = nc.dram_tensor("pack", [R_TOT, D], BF16, kind="ExternalInput")
    out_d = nc.dram_tensor("out", [S, D], BF16, kind="ExternalOutput")

    with tile.TileContext(nc) as tc:
        const = tc.alloc_tile_pool(name="const", bufs=1)
        big = tc.alloc_tile_pool(name="big", bufs=1)
        stage = tc.alloc_tile_pool(name="stage", bufs=4)
        dram = tc.alloc_tile_pool(name="dram", bufs=1, space="DRAM")
        psum_tr = tc.alloc_tile_pool(name="psum_tr", bufs=3, space="PSUM")
        psum_gate = tc.alloc_tile_pool(name="psum_gate", bufs=2, space="PSUM")

        # --- AllGather the We shards into full We (HBM->HBM), first thing so
        # it overlaps the x prep below. Collectives can't touch I/O tensors,
        # so bounce through Internal dram tiles.
        we_in_b = dram.tile([WE_SHARD, D], BF16)
        we_all_b = dram.tile([E * D, D], BF16, addr_space="Shared")
        nc.gpsimd.dma_start(out=we_in_b, in_=pack_d[R_WE : R_WE + WE_SHARD, :])
        nc.gpsimd.collective_compute(
            "AllGather",
            ALU.bypass,
            replica_groups=[list(range(B))],
            ins=[we_in_b.opt()],
            outs=[we_all_b.opt()],
        )

        ident = const.tile([P, P], BF16)
        make_identity(nc, ident)
        ones_col_f = const.tile([P, 1], F32)
        nc.vector.memset(ones_col_f, 1.0)
        ones_row_f = const.tile([1, P], F32)
        nc.vector.memset(ones_row_f, 1.0)

        # --- gate weights: load Wg rows, PE-transpose to [din, e] ---
        wg_sb = const.tile([E, D], BF16)
        nc.sync.dma_start(out=wg_sb, in_=pack_d[R_WG : R_WG + E, :])
        pwg = psum_gate.tile([P, NK, E], BF16, tag="pwg", bufs=1)
        for k in range(NK):
            nc.tensor.matmul(
                pwg[:, k, :],
                wg_sb[0:E, k * P : (k + 1) * P],
                ident[0:E, 0:E],
                is_transpose=True,
                start=True,
                stop=True,
            )
        wgT = const.tile([P, NK, E], BF16)
        nc.scalar.copy(wgT, pwg)

        if use_bg:
            bg_bc = const.tile([P, E], BF16)
            nc.gpsimd.dma_start(
                out=bg_bc,
                in_=bass.AP(tensor=pack_d, offset=R_BG * D, ap=[[0, P], [1, E]]),
            )
        if use_be:
            be_bf = const.tile([E, D], BF16)
            nc.sync.dma_start(out=be_bf, in_=pack_d[R_BE : R_BE + E, :])
            ones_row_bf = const.tile([1, P], BF16)
            nc.vector.memset(ones_row_bf, 1.0)

        # --- persistent transposed operands ---
        xT = big.tile([P, NK, NS, P], BF16)    # 32 KB/partition
        weT = big.tile([P, E, NK, D], BF16)    # 64 KB/partition
        gate_all = const.tile([P, NS, E], F32)

        # --- x prep: load bf16, PE-transpose ---
        for st in range(NS):
            x_nat = stage.tile([P, D], BF16, tag="stg")
            nc.sync.dma_start(out=x_nat, in_=pack_d[st * P : (st + 1) * P, :])
            ptr = psum_tr.tile([P, NK, P], BF16, tag="ptr")
            for k in range(NK):
                nc.tensor.matmul(
                    ptr[:, k, :],
                    x_nat[:, k * P : (k + 1) * P],
                    ident,
                    is_transpose=True,
                    start=True,
                    stop=True,
                )
            nc.scalar.copy(xT[:, :, st, :], ptr)

        # --- We prep: load bf16 from the gathered buffer, PE-transpose ---
        for e in range(E):
            for dc in range(NK):  # 8 dout-chunks of 128 rows
                r0 = e * D + dc * P
                we_nat = stage.tile([P, D], BF16, tag="stg")
                nc.sync.dma_start(out=we_nat, in_=we_all_b[r0 : r0 + P, :])
                ptr = psum_tr.tile([P, NK, P], BF16, tag="ptr")
                for k in range(NK):
                    nc.tensor.matmul(
                        ptr[:, k, :],
                        we_nat[:, k * P : (k + 1) * P],
                        ident,
                        is_transpose=True,
                        start=True,
                        stop=True,
                    )
                nc.vector.tensor_copy(weT[:, e, :, dc * P : (dc + 1) * P], ptr)

        # --- gate matmuls: psum += xT.T @ WgT over 8 k-chunks ---
        for st in range(NS):
            pg = psum_gate.tile([P, E], F32, tag="pg")
            for k in range(NK):
                nc.tensor.matmul(
                    pg, xT[:, k, st, :], wgT[:, k, :],
                    start=(k == 0), stop=(k == NK - 1),
                )
            if use_bg:
                nc.vector.tensor_add(gate_all[:, st, :], pg, bg_bc)
            else:
                nc.scalar.copy(gate_all[:, st, :], pg)

        # --- counts -> weights (broadcast to all partitions) ---
        rowmax = const.tile([P, NS], F32)
        nc.vector.tensor_reduce(rowmax, gate_all, axis=mybir.AxisListType.X, op=ALU.max)
        ismax = const.tile([P, E, NS], F32)
        g_ens = gate_all.rearrange("p n e -> p e n")
        rm_bc = bass.AP(
            tensor=rowmax.tensor, offset=rowmax.offset,
            ap=[rowmax.ap[0], [0, E], [1, NS]],
        )
        nc.vector.tensor_tensor(ismax, g_ens, rm_bc, op=ALU.is_ge)
        counts_part = const.tile([P, E], F32)
        nc.vector.tensor_reduce(
            counts_part, ismax, axis=mybir.AxisListType.X, op=ALU.add
        )

        pc1 = psum_gate.tile([1, E], F32, tag="pc1", bufs=1)
        nc.tensor.matmul(pc1, ones_col_f, counts_part, start=True, stop=True)
        counts_sb = const.tile([1, E], F32)
        nc.scalar.copy(counts_sb, pc1)
        pc2 = psum_gate.tile([P, E], F32, tag="pc2", bufs=1)
        nc.tensor.matmul(pc2, ones_row_f, counts_sb, start=True, stop=True)
        w_bc = const.tile([P, E], F32)
        nc.scalar.mul(w_bc, pc2, 1.0 / S)

        psum_gate.release()
        psum_tr.release()

        # --- main expert matmuls + fused epilogue ---
        psum_main = tc.alloc_tile_pool(name="psum_main", bufs=4, space="PSUM")
        relu_p = tc.alloc_tile_pool(name="relu_p", bufs=6)
        acc_p = tc.alloc_tile_pool(name="acc_p", bufs=4)
        out_p = tc.alloc_tile_pool(name="out_p", bufs=3)

        for st in range(NS):
            accs = []
            for half in range(2):
                pts = [
                    psum_main.tile([P, D], F32, tag="pm", name=f"pm{e2}")
                    for e2 in range(2)
                ]
                if use_be:
                    for e2, pt in enumerate(pts):
                        e = half * 2 + e2
                        for c in range(ND):
                            nc.tensor.matmul(
                                pt[:, c * NC : (c + 1) * NC],
                                ones_row_bf,
                                be_bf[e : e + 1, c * NC : (c + 1) * NC],
                                start=True, stop=False,
                            )
                for k in range(NK):
                    lhs = xT[:, k, st, :]
                    for e2, pt in enumerate(pts):
                        for c in range(ND):
                            e = half * 2 + e2
                            nc.tensor.matmul(
                                pt[:, c * NC : (c + 1) * NC],
                                lhs,
                                weT[:, e, k, c * NC : (c + 1) * NC],
                                start=(k == 0 and not use_be),
                                stop=(k == NK - 1),
                            )
                trs = []
                for e2, pt in enumerate(pts):
                    e = half * 2 + e2
                    tr = relu_p.tile([P, D], BF16, tag="tr")
                    nc.scalar.activation(tr, pt, RELU, scale=w_bc[:, e : e + 1])
                    trs.append(tr)
                acc = acc_p.tile([P, D], F32, tag="acc")
                nc.vector.tensor_add(acc, trs[0], trs[1])
                accs.append(acc)
            o = out_p.tile([P, D], BF16, tag="o")
            nc.vector.tensor_add(o, accs[0], accs[1])
            nc.sync.dma_start(out=out_d[st * P : (st + 1) * P, :], in_=o)

        out_p.release()
        acc_p.release()
        relu_p.release()
        psum_main.release()
        stage.release()
        dram.release()
        big.release()
        const.release()

    _split_sync_waits(nc)
    return nc


_CACHE = {}


def _get_kernel(use_bg: bool, use_be: bool) -> bass.Bass:
    key = (use_bg, use_be)
    if key not in _CACHE:
        _CACHE[key] = build_kernel(use_bg, use_be)
    return _CACHE[key]


def kernel(x, Wg, bg, We, be, _trace=False):
    x = np.asarray(x, dtype=np.float32)
    Wg = np.asarray(Wg, dtype=np.float32)
    bg = np.asarray(bg, dtype=np.float32)
    We = np.asarray(We, dtype=np.float32)
    be = np.asarray(be, dtype=np.float32)
    assert x.shape == (B, S, D) and Wg.shape == (E, D)
    assert We.shape == (E, D, D) and bg.shape == (E,) and be.shape == (E, D)

    use_bg = bool(np.any(bg))
    use_be = bool(np.any(be))
    nc = _get_kernel(use_bg, use_be)

    # host-side bf16 cast + pack: one input tensor per core, We sharded
    x_bf = x.astype(NPBF16)
    we_rows = We.reshape(E * D, D).astype(NPBF16)
    tail = np.empty((R_TOT - R_WG, D), dtype=NPBF16)
    tail[0:E] = Wg.astype(NPBF16)
    tail[E] = 0
    tail[E, 0:E] = bg.astype(NPBF16)
    tail[E + 1 : E + 1 + E] = be.astype(NPBF16)

    in_maps = [
        {
            "pack": np.concatenate(
                [x_bf[b], we_rows[b * WE_SHARD : (b + 1) * WE_SHARD], tail],
                axis=0,
            )
        }
        for b in range(B)
    ]
    try:
        res = run_bass_kernel_spmd(
            nc, in_maps, core_ids=list(range(B)), trace=_trace
        )
    except ModuleNotFoundError:
        # NTFF profile hook unavailable in this container; run untraced
        res = run_bass_kernel_spmd(nc, in_maps, core_ids=list(range(B)))
    out = np.stack([res.results[b]["out"] for b in range(B)], axis=0).astype(
        np.float32
    )
    if _trace:
        return out, res
    return out


# revision 13
# speedup vs baseline: 3.4663x; 1.2057x over previous
"""MoE routing kernel for Trainium2 (8 NeuronCores, data-parallel over batch).

Problem: x[B=8,S=2048,D=1024] f32; gate Wg[E=4,D]+bg; experts We[E,D,D]+be.
  gate = x @ Wg.T + bg; top1 = argmax(gate); weights[b,e] = count_e(top1[b])/S
  out[b] = sum_e weights[b,e] * relu(x[b] @ We[e].T + be[e])

The warm-path cost in this environment is dominated by host<->device traffic
over the axon tunnel (~38 MB/s), not device compute (~0.3 ms). So the design
minimizes bytes moved:
  - everything is cast to bf16 on the host (x, We, Wg, bg, be) and packed into
    ONE [6153, 1024] bf16 dram tensor per core (one transfer, low latency);
  - the device output is bf16 [S, D], upcast to f32 on the host;
  - measured end-to-end rel err of the full-bf16 pipeline: 5.3e-3 (tolerance
    2e-2; routing argmax from bf16 inputs flips only ~15/16384 tokens).

Sharding: batch dim across the 8 cores (1 batch element per core); expert
weights replicated. No collectives; host gathers per-core outputs.

Per-core kernel structure:
  - PE-transpose x and We tiles so the contraction dim (din) lands on
    partitions; all matmuls in bf16 (PE 1 cyc/row), f32 PSUM accumulation.
  - gate = xT.T @ WgT accumulated over 8 k-chunks in one PSUM tile, then
    argmax->counts->weights entirely on-chip (is_ge + reductions + two tiny
    f32 matmuls for partition-sum and partition-broadcast).
  - expert matmuls: K=1024 contracted in 8 chunks accumulating in PSUM,
    N=512 per matmul (one PSUM bank).
  - epilogue: relu(w_e * y) on ScalarE (w_e >= 0 so the weight folds into the
    activation scale) + DVE add tree, final add emits bf16 out tile.
"""

import numpy as np
import ml_dtypes

import jax

# Persist XLA executables across processes/calls: run_bass_via_pjrt re-jits a
# fresh closure every call, so without this each warm call pays ~0.5s of
# XLA+BIR recompile. Harmless no-op if the PJRT client can't serialize.
jax.config.update("jax_compilation_cache_dir", "/tmp/jax_comp_cache")
jax.config.update("jax_persistent_cache_min_compile_time_secs", 0.0)
jax.config.update("jax_persistent_cache_min_entry_size_bytes", 0)

import concourse.bass as bass
import concourse.tile as tile
from concourse import mybir
from concourse.bass_utils import run_bass_kernel_spmd
from concourse.masks import make_identity
from concourse.vector_clock import ScopedClock, VectorClock

F32 = mybir.dt.float32
BF16 = mybir.dt.bfloat16
RELU = mybir.ActivationFunctionType.Relu
ALU = mybir.AluOpType
NPBF16 = ml_dtypes.bfloat16

B, S, D, E = 8, 2048, 1024, 4
P = 128
NS = S // P   # 16 s-tiles
NK = D // P   # 8 contraction chunks
NC = 512      # matmul moving free dim (one PSUM bank of f32)
ND = D // NC  # 2 dout chunks

# pack row layout (rows of D bf16 values). We is sharded 1/8th per core and
# AllGathered on device — 8MB crosses the slow host tunnel once, not 8 times.
WE_SHARD = E * D // B  # 512 rows per core
R_X = 0                # x[b]:  S rows
R_WE = R_X + S         # We shard: rows [c*512, (c+1)*512) of We.reshape(E*D, D)
R_WG = R_WE + WE_SHARD # Wg:    E rows
R_BG = R_WG + E        # bg:    1 row (first E entries)
R_BE = R_BG + 1        # be:    E rows
R_TOT = R_BE + E       # 2569


def _apply_tile_drain_patch():
    """The walrus build in this container only encodes one sync-wait on a
    CTRL instruction; Tile's kernel-tail drain attaches one wait per active
    proc to a single InstDrain and fails codegen. Split it into one drain
    per proc instead."""
    if getattr(tile.TileContext, "_moe_drain_patch", False):
        return
    tile.TileContext._moe_drain_patch = True

    def _drain_and_barrier(self, tick_clock, wait_clock):
        gc = tick_clock.global_clock
        scopes = [(None, gc)] if isinstance(gc, VectorClock) else gc.items()
        n_emitted = 0
        for scope, vc in scopes:
            n = len(vc)
            for proc in range(n):
                t = vc[proc]
                if t > 0:
                    single = VectorClock([t if i == proc else 0 for i in range(n)])
                    d = self.nc.sync.drain()
                    wait_clock.add_sem_waits(d.ins, ScopedClock({scope: single}))
                    n_emitted += 1
        if n_emitted == 0:
            self.nc.sync.drain()
        self.nc.all_engine_barrier()
        popped = self.nc._tile_sem_poison_stack.pop()
        assert popped is self._sem_poison
        self.nc.clear_and_free_semaphores(list(self.sems.allocated().values()))
        self.nc.all_engine_barrier()

    tile.TileContext._drain_and_barrier = _drain_and_barrier


_apply_tile_drain_patch()


def _split_sync_waits(nc: bass.Bass, limit: int = 1):
    """This container's walrus encodes at most one sync-wait per instruction.
    Hoist excess waits onto same-engine NoOps emitted immediately before the
    instruction — the engine stream blocks on each in turn, which is
    semantically identical to waiting on all of them at once."""
    ctr = 0
    for f in nc.m.functions:
        for bb in f.blocks:
            insts = list(bb.instructions)
            out = []
            changed = False
            for ins in insts:
                si = ins.sync_info
                waits = list(si.on_wait) if si is not None else []
                if len(waits) > limit:
                    changed = True
                    for w in waits[:-limit]:
                        ctr += 1
                        nop = mybir.InstNoOp(name=f"wsplit-{ctr}", ins=[], outs=[])
                        nop.engine = ins.engine
                        nop.sync_info = mybir.SyncInfo(on_wait=[w], on_update=[])
                        out.append(nop)
                    ins.sync_info = mybir.SyncInfo(
                        on_wait=waits[-limit:], on_update=list(si.on_update)
                    )
                out.append(ins)
            if changed:
                bb.instructions = out


def build_kernel(use_bg: bool, use_be: bool) -> bass.Bass:
    nc = bass.Bass()
    pack_d = nc.dram_tensor("pack", [R_TOT, D], BF16, kind="ExternalInput")
    # out is uint8 with a per-token f32 scale: out_f32 ~= out / out_s. The
    # device quantizes q = round(y * qs) with qs = 255/(rowmax+eps) and ships
    # qs itself, so host dequant by 1/qs cancels any reciprocal error exactly.
    # Halves the d2h bytes vs bf16 at the same global error (<= rowmax/510).
    out_d = nc.dram_tensor("out", [S, D], mybir.dt.uint8, kind="ExternalOutput")
    outs_d = nc.dram_tensor("out_s", [S, 1], F32, kind="ExternalOutput")

    with tile.TileContext(nc) as tc:
        const = tc.alloc_tile_pool(name="const", bufs=1)
        big = tc.alloc_tile_pool(name="big", bufs=1)
        stage = tc.alloc_tile_pool(name="stage", bufs=4)
        dram = tc.alloc_tile_pool(name="dram", bufs=1, space="DRAM")
        psum_tr = tc.alloc_tile_pool(name="psum_tr", bufs=3, space="PSUM")
        psum_gate = tc.alloc_tile_pool(name="psum_gate", bufs=2, space="PSUM")

        # --- AllGather the We shards into full We (HBM->HBM), first thing so
        # it overlaps the x prep below. Collectives can't touch I/O tensors,
        # so bounce through Internal dram tiles.
        we_in_b = dram.tile([WE_SHARD, D], BF16)
        we_all_b = dram.tile([E * D, D], BF16, addr_space="Shared")
        nc.gpsimd.dma_start(out=we_in_b, in_=pack_d[R_WE : R_WE + WE_SHARD, :])
        nc.gpsimd.collective_compute(
            "AllGather",
            ALU.bypass,
            replica_groups=[list(range(B))],
            ins=[we_in_b.opt()],
            outs=[we_all_b.opt()],
        )

        ident = const.tile([P, P], BF16)
        make_identity(nc, ident)
        ones_col_f = const.tile([P, 1], F32)
        nc.vector.memset(ones_col_f, 1.0)
        ones_row_f = const.tile([1, P], F32)
        nc.vector.memset(ones_row_f, 1.0)

        # --- gate weights: load Wg rows, PE-transpose to [din, e] ---
        wg_sb = const.tile([E, D], BF16)
        nc.sync.dma_start(out=wg_sb, in_=pack_d[R_WG : R_WG + E, :])
        pwg = psum_gate.tile([P, NK, E], BF16, tag="pwg", bufs=1)
        for k in range(NK):
            nc.tensor.matmul(
                pwg[:, k, :],
                wg_sb[0:E, k * P : (k + 1) * P],
                ident[0:E, 0:E],
                is_transpose=True,
                start=True,
                stop=True,
            )
        wgT = const.tile([P, NK, E], BF16)
        nc.scalar.copy(wgT, pwg)

        if use_bg:
            bg_bc = const.tile([P, E], BF16)
            nc.gpsimd.dma_start(
                out=bg_bc,
                in_=bass.AP(tensor=pack_d, offset=R_BG * D, ap=[[0, P], [1, E]]),
            )
        if use_be:
            be_bf = const.tile([E, D], BF16)
            nc.sync.dma_start(out=be_bf, in_=pack_d[R_BE : R_BE + E, :])
            ones_row_bf = const.tile([1, P], BF16)
            nc.vector.memset(ones_row_bf, 1.0)

        # --- persistent transposed operands ---
        xT = big.tile([P, NK, NS, P], BF16)    # 32 KB/partition
        weT = big.tile([P, E, NK, D], BF16)    # 64 KB/partition
        gate_all = const.tile([P, NS, E], F32)

        # --- x prep: load bf16, PE-transpose ---
        for st in range(NS):
            x_nat = stage.tile([P, D], BF16, tag="stg")
            nc.sync.dma_start(out=x_nat, in_=pack_d[st * P : (st + 1) * P, :])
            ptr = psum_tr.tile([P, NK, P], BF16, tag="ptr")
            for k in range(NK):
                nc.tensor.matmul(
                    ptr[:, k, :],
                    x_nat[:, k * P : (k + 1) * P],
                    ident,
                    is_transpose=True,
                    start=True,
                    stop=True,
                )
            nc.scalar.copy(xT[:, :, st, :], ptr)

        # --- We prep: load bf16 from the gathered buffer, PE-transpose ---
        for e in range(E):
            for dc in range(NK):  # 8 dout-chunks of 128 rows
                r0 = e * D + dc * P
                we_nat = stage.tile([P, D], BF16, tag="stg")
                nc.sync.dma_start(out=we_nat, in_=we_all_b[r0 : r0 + P, :])
                ptr = psum_tr.tile([P, NK, P], BF16, tag="ptr")
                for k in range(NK):
                    nc.tensor.matmul(
                        ptr[:, k, :],
                        we_nat[:, k * P : (k + 1) * P],
                        ident,
                        is_transpose=True,
                        start=True,
                        stop=True,
                    )
                nc.vector.tensor_copy(weT[:, e, :, dc * P : (dc + 1) * P], ptr)

        # --- gate matmuls: psum += xT.T @ WgT over 8 k-chunks ---
        for st in range(NS):
            pg = psum_gate.tile([P, E], F32, tag="pg")
            for k in range(NK):
                nc.tensor.matmul(
                    pg, xT[:, k, st, :], wgT[:, k, :],
                    start=(k == 0), stop=(k == NK - 1),
                )
            if use_bg:
                nc.vector.tensor_add(gate_all[:, st, :], pg, bg_bc)
            else:
                nc.scalar.copy(gate_all[:, st, :], pg)

        # --- counts -> weights (broadcast to all partitions) ---
        rowmax = const.tile([P, NS], F32)
        nc.vector.tensor_reduce(rowmax, gate_all, axis=mybir.AxisListType.X, op=ALU.max)
        ismax = const.tile([P, E, NS], F32)
        g_ens = gate_all.rearrange("p n e -> p e n")
        rm_bc = bass.AP(
            tensor=rowmax.tensor, offset=rowmax.offset,
            ap=[rowmax.ap[0], [0, E], [1, NS]],
        )
        nc.vector.tensor_tensor(ismax, g_ens, rm_bc, op=ALU.is_ge)
        counts_part = const.tile([P, E], F32)
        nc.vector.tensor_reduce(
            counts_part, ismax, axis=mybir.AxisListType.X, op=ALU.add
        )

        pc1 = psum_gate.tile([1, E], F32, tag="pc1", bufs=1)
        nc.tensor.matmul(pc1, ones_col_f, counts_part, start=True, stop=True)
        counts_sb = const.tile([1, E], F32)
        nc.scalar.copy(counts_sb, pc1)
        pc2 = psum_gate.tile([P, E], F32, tag="pc2", bufs=1)
        nc.tensor.matmul(pc2, ones_row_f, counts_sb, start=True, stop=True)
        w_bc = const.tile([P, E], F32)
        nc.scalar.mul(w_bc, pc2, 1.0 / S)

        psum_gate.release()
        psum_tr.release()

        # --- main expert matmuls + fused epilogue ---
        psum_main = tc.alloc_tile_pool(name="psum_main", bufs=4, space="PSUM")
        relu_p = tc.alloc_tile_pool(name="relu_p", bufs=6)
        acc_p = tc.alloc_tile_pool(name="acc_p", bufs=4)
        out_p = tc.alloc_tile_pool(name="out_p", bufs=3)
        q_p = tc.alloc_tile_pool(name="q_p", bufs=3)
        MAGIC = 8388608.0  # 2^23: x + MAGIC - MAGIC == round-to-nearest-even(x)

        for st in range(NS):
            accs = []
            for half in range(2):
                pts = [
                    psum_main.tile([P, D], F32, tag="pm", name=f"pm{e2}")
                    for e2 in range(2)
                ]
                if use_be:
                    for e2, pt in enumerate(pts):
                        e = half * 2 + e2
                        for c in range(ND):
                            nc.tensor.matmul(
                                pt[:, c * NC : (c + 1) * NC],
                                ones_row_bf,
                                be_bf[e : e + 1, c * NC : (c + 1) * NC],
                                start=True, stop=False,
                            )
                for k in range(NK):
                    lhs = xT[:, k, st, :]
                    for e2, pt in enumerate(pts):
                        for c in range(ND):
                            e = half * 2 + e2
                            nc.tensor.matmul(
                                pt[:, c * NC : (c + 1) * NC],
                                lhs,
                                weT[:, e, k, c * NC : (c + 1) * NC],
                                start=(k == 0 and not use_be),
                                stop=(k == NK - 1),
                            )
                trs = []
                for e2, pt in enumerate(pts):
                    e = half * 2 + e2
                    tr = relu_p.tile([P, D], F32, tag="tr")
                    nc.scalar.activation(tr, pt, RELU, scale=w_bc[:, e : e + 1])
                    trs.append(tr)
                acc = acc_p.tile([P, D], F32, tag="acc")
                nc.vector.tensor_add(acc, trs[0], trs[1])
                accs.append(acc)
            o_f = out_p.tile([P, D], F32, tag="o")
            nc.vector.tensor_add(o_f, accs[0], accs[1])
            # per-token uint8 quantization (y >= 0 post relu-sum)
            rmax = q_p.tile([P, 1], F32, tag="rmax")
            nc.vector.tensor_reduce(rmax, o_f, axis=mybir.AxisListType.X, op=ALU.max)
            qden = q_p.tile([P, 1], F32, tag="qden")
            nc.vector.tensor_scalar(
                qden, rmax, 1e-30, 1.0 / 255.0, op0=ALU.add, op1=ALU.mult
            )
            qs = q_p.tile([P, 1], F32, tag="qs")
            nc.vector.reciprocal(qs, qden)
            q_r = out_p.tile([P, D], F32, tag="qr")
            nc.vector.tensor_scalar(
                q_r, o_f, qs[:, 0:1], MAGIC, op0=ALU.mult, op1=ALU.add
            )
            q_u8 = q_p.tile([P, D], mybir.dt.uint8, tag="q8")
            nc.vector.tensor_scalar(q_u8, q_r, MAGIC, None, op0=ALU.subtract)
            nc.sync.dma_start(out=out_d[st * P : (st + 1) * P, :], in_=q_u8)
            nc.sync.dma_start(out=outs_d[st * P : (st + 1) * P, :], in_=qs)

        q_p.release()
        out_p.release()
        acc_p.release()
        relu_p.release()
        psum_main.release()
        stage.release()
        dram.release()
        big.release()
        const.release()

    _split_sync_waits(nc)
    return nc


_CACHE = {}


def _get_kernel(use_bg: bool, use_be: bool) -> bass.Bass:
    key = (use_bg, use_be)
    if key not in _CACHE:
        _CACHE[key] = build_kernel(use_bg, use_be)
    return _CACHE[key]


def kernel(x, Wg, bg, We, be, _trace=False):
    x = np.asarray(x, dtype=np.float32)
    Wg = np.asarray(Wg, dtype=np.float32)
    bg = np.asarray(bg, dtype=np.float32)
    We = np.asarray(We, dtype=np.float32)
    be = np.asarray(be, dtype=np.float32)
    assert x.shape == (B, S, D) and Wg.shape == (E, D)
    assert We.shape == (E, D, D) and bg.shape == (E,) and be.shape == (E, D)

    use_bg = bool(np.any(bg))
    use_be = bool(np.any(be))
    nc = _get_kernel(use_bg, use_be)

    # host-side bf16 cast + pack: one input tensor per core, We sharded
    x_bf = x.astype(NPBF16)
    we_rows = We.reshape(E * D, D).astype(NPBF16)
    tail = np.empty((R_TOT - R_WG, D), dtype=NPBF16)
    tail[0:E] = Wg.astype(NPBF16)
    tail[E] = 0
    tail[E, 0:E] = bg.astype(NPBF16)
    tail[E + 1 : E + 1 + E] = be.astype(NPBF16)

    in_maps = [
        {
            "pack": np.concatenate(
                [x_bf[b], we_rows[b * WE_SHARD : (b + 1) * WE_SHARD], tail],
                axis=0,
            )
        }
        for b in range(B)
    ]
    try:
        res = run_bass_kernel_spmd(
            nc, in_maps, core_ids=list(range(B)), trace=_trace
        )
    except ModuleNotFoundError:
        # NTFF profile hook unavailable in this container; run untraced
        res = run_bass_kernel_spmd(nc, in_maps, core_ids=list(range(B)))
    q = np.stack([res.results[b]["out"] for b in range(B)], axis=0)
    qs = np.stack([res.results[b]["out_s"] for b in range(B)], axis=0)
    out = q.astype(np.float32) * (1.0 / qs)
    if _trace:
        return out, res
    return out


# revision 16
# speedup vs baseline: 4.2504x; 1.2262x over previous
"""MoE routing kernel for Trainium2 (8 NeuronCores, data-parallel over batch).

Problem: x[B=8,S=2048,D=1024] f32; gate Wg[E=4,D]+bg; experts We[E,D,D]+be.
  gate = x @ Wg.T + bg; top1 = argmax(gate); weights[b,e] = count_e(top1[b])/S
  out[b] = sum_e weights[b,e] * relu(x[b] @ We[e].T + be[e])

The warm-path cost in this environment is dominated by host<->device traffic
over the axon tunnel (~38 MB/s), not device compute (~0.3 ms). So the design
minimizes bytes moved:
  - everything is cast to bf16 on the host (x, We, Wg, bg, be) and packed into
    ONE [6153, 1024] bf16 dram tensor per core (one transfer, low latency);
  - the device output is bf16 [S, D], upcast to f32 on the host;
  - measured end-to-end rel err of the full-bf16 pipeline: 5.3e-3 (tolerance
    2e-2; routing argmax from bf16 inputs flips only ~15/16384 tokens).

Sharding: batch dim across the 8 cores (1 batch element per core); expert
weights replicated. No collectives; host gathers per-core outputs.

Per-core kernel structure:
  - PE-transpose x and We tiles so the contraction dim (din) lands on
    partitions; all matmuls in bf16 (PE 1 cyc/row), f32 PSUM accumulation.
  - gate = xT.T @ WgT accumulated over 8 k-chunks in one PSUM tile, then
    argmax->counts->weights entirely on-chip (is_ge + reductions + two tiny
    f32 matmuls for partition-sum and partition-broadcast).
  - expert matmuls: K=1024 contracted in 8 chunks accumulating in PSUM,
    N=512 per matmul (one PSUM bank).
  - epilogue: relu(w_e * y) on ScalarE (w_e >= 0 so the weight folds into the
    activation scale) + DVE add tree, final add emits bf16 out tile.
"""

from concurrent.futures import ThreadPoolExecutor

import numpy as np
import ml_dtypes

import jax

# Persist XLA executables across processes/calls: run_bass_via_pjrt re-jits a
# fresh closure every call, so without this each warm call pays ~0.5s of
# XLA+BIR recompile. Harmless no-op if the PJRT client can't serialize.
jax.config.update("jax_compilation_cache_dir", "/tmp/jax_comp_cache")
jax.config.update("jax_persistent_cache_min_compile_time_secs", 0.0)
jax.config.update("jax_persistent_cache_min_entry_size_bytes", 0)

import concourse.bass as bass
import concourse.tile as tile
from concourse import mybir
from concourse.bass_utils import run_bass_kernel_spmd
from concourse.masks import make_identity
from concourse.vector_clock import ScopedClock, VectorClock

F32 = mybir.dt.float32
BF16 = mybir.dt.bfloat16
RELU = mybir.ActivationFunctionType.Relu
ALU = mybir.AluOpType
NPBF16 = ml_dtypes.bfloat16

B, S, D, E = 8, 2048, 1024, 4
P = 128
NS = S // P   # 16 s-tiles
NK = D // P   # 8 contraction chunks
NC = 512      # matmul moving free dim (one PSUM bank of f32)
ND = D // NC  # 2 dout chunks

# pack row layout (rows of D bf16 values). We is sharded 1/8th per core and
# AllGathered on device — 8MB crosses the slow host tunnel once, not 8 times.
WE_SHARD = E * D // B  # 512 rows per core
R_X = 0                # x[b]:  S rows
R_WE = R_X + S         # We shard: rows [c*512, (c+1)*512) of We.reshape(E*D, D)
R_WG = R_WE + WE_SHARD # Wg:    E rows
R_BG = R_WG + E        # bg:    1 row (first E entries)
R_BE = R_BG + 1        # be:    E rows
R_TOT = R_BE + E       # 2569


def _apply_tile_drain_patch():
    """The walrus build in this container only encodes one sync-wait on a
    CTRL instruction; Tile's kernel-tail drain attaches one wait per active
    proc to a single InstDrain and fails codegen. Split it into one drain
    per proc instead."""
    if getattr(tile.TileContext, "_moe_drain_patch", False):
        return
    tile.TileContext._moe_drain_patch = True

    def _drain_and_barrier(self, tick_clock, wait_clock):
        gc = tick_clock.global_clock
        scopes = [(None, gc)] if isinstance(gc, VectorClock) else gc.items()
        n_emitted = 0
        for scope, vc in scopes:
            n = len(vc)
            for proc in range(n):
                t = vc[proc]
                if t > 0:
                    single = VectorClock([t if i == proc else 0 for i in range(n)])
                    d = self.nc.sync.drain()
                    wait_clock.add_sem_waits(d.ins, ScopedClock({scope: single}))
                    n_emitted += 1
        if n_emitted == 0:
            self.nc.sync.drain()
        self.nc.all_engine_barrier()
        popped = self.nc._tile_sem_poison_stack.pop()
        assert popped is self._sem_poison
        self.nc.clear_and_free_semaphores(list(self.sems.allocated().values()))
        self.nc.all_engine_barrier()

    tile.TileContext._drain_and_barrier = _drain_and_barrier


_apply_tile_drain_patch()


def _split_sync_waits(nc: bass.Bass, limit: int = 1):
    """This container's walrus encodes at most one sync-wait per instruction.
    Hoist excess waits onto same-engine NoOps emitted immediately before the
    instruction — the engine stream blocks on each in turn, which is
    semantically identical to waiting on all of them at once."""
    ctr = 0
    for f in nc.m.functions:
        for bb in f.blocks:
            insts = list(bb.instructions)
            out = []
            changed = False
            for ins in insts:
                si = ins.sync_info
                waits = list(si.on_wait) if si is not None else []
                if len(waits) > limit:
                    changed = True
                    for w in waits[:-limit]:
                        ctr += 1
                        nop = mybir.InstNoOp(name=f"wsplit-{ctr}", ins=[], outs=[])
                        nop.engine = ins.engine
                        nop.sync_info = mybir.SyncInfo(on_wait=[w], on_update=[])
                        out.append(nop)
                    ins.sync_info = mybir.SyncInfo(
                        on_wait=waits[-limit:], on_update=list(si.on_update)
                    )
                out.append(ins)
            if changed:
                bb.instructions = out


def build_kernel(use_bg: bool, use_be: bool) -> bass.Bass:
    nc = bass.Bass()
    pack_d = nc.dram_tensor("pack", [R_TOT, D], BF16, kind="ExternalInput")
    # out is uint8 with a per-token f32 scale: out_f32 ~= out / out_s. The
    # device quantizes q = round(y * qs) with qs = 255/(rowmax+eps) and ships
    # qs itself, so host dequant by 1/qs cancels any reciprocal error exactly.
    # Halves the d2h bytes vs bf16 at the same global error (<= rowmax/510).
    out_d = nc.dram_tensor("out", [S, D], mybir.dt.uint8, kind="ExternalOutput")
    outs_d = nc.dram_tensor("out_s", [S, 1], F32, kind="ExternalOutput")

    with tile.TileContext(nc) as tc:
        const = tc.alloc_tile_pool(name="const", bufs=1)
        big = tc.alloc_tile_pool(name="big", bufs=1)
        stage = tc.alloc_tile_pool(name="stage", bufs=4)
        dram = tc.alloc_tile_pool(name="dram", bufs=1, space="DRAM")
        psum_tr = tc.alloc_tile_pool(name="psum_tr", bufs=3, space="PSUM")
        psum_gate = tc.alloc_tile_pool(name="psum_gate", bufs=2, space="PSUM")

        # --- AllGather the We shards into full We (HBM->HBM), first thing so
        # it overlaps the x prep below. Collectives can't touch I/O tensors,
        # so bounce through Internal dram tiles.
        we_in_b = dram.tile([WE_SHARD, D], BF16)
        we_all_b = dram.tile([E * D, D], BF16, addr_space="Shared")
        nc.gpsimd.dma_start(out=we_in_b, in_=pack_d[R_WE : R_WE + WE_SHARD, :])
        nc.gpsimd.collective_compute(
            "AllGather",
            ALU.bypass,
            replica_groups=[list(range(B))],
            ins=[we_in_b.opt()],
            outs=[we_all_b.opt()],
        )

        ident = const.tile([P, P], BF16)
        make_identity(nc, ident)
        ones_col_f = const.tile([P, 1], F32)
        nc.vector.memset(ones_col_f, 1.0)
        ones_row_f = const.tile([1, P], F32)
        nc.vector.memset(ones_row_f, 1.0)

        # --- gate weights: load Wg rows, PE-transpose to [din, e] ---
        wg_sb = const.tile([E, D], BF16)
        nc.sync.dma_start(out=wg_sb, in_=pack_d[R_WG : R_WG + E, :])
        pwg = psum_gate.tile([P, NK, E], BF16, tag="pwg", bufs=1)
        for k in range(NK):
            nc.tensor.matmul(
                pwg[:, k, :],
                wg_sb[0:E, k * P : (k + 1) * P],
                ident[0:E, 0:E],
                is_transpose=True,
                start=True,
                stop=True,
            )
        wgT = const.tile([P, NK, E], BF16)
        nc.scalar.copy(wgT, pwg)

        if use_bg:
            bg_bc = const.tile([P, E], BF16)
            nc.gpsimd.dma_start(
                out=bg_bc,
                in_=bass.AP(tensor=pack_d, offset=R_BG * D, ap=[[0, P], [1, E]]),
            )
        if use_be:
            be_bf = const.tile([E, D], BF16)
            nc.sync.dma_start(out=be_bf, in_=pack_d[R_BE : R_BE + E, :])
            ones_row_bf = const.tile([1, P], BF16)
            nc.vector.memset(ones_row_bf, 1.0)

        # --- persistent transposed operands ---
        xT = big.tile([P, NK, NS, P], BF16)    # 32 KB/partition
        weT = big.tile([P, E, NK, D], BF16)    # 64 KB/partition
        gate_all = const.tile([P, NS, E], F32)

        # --- x prep: load bf16, PE-transpose ---
        for st in range(NS):
            x_nat = stage.tile([P, D], BF16, tag="stg")
            nc.sync.dma_start(out=x_nat, in_=pack_d[st * P : (st + 1) * P, :])
            ptr = psum_tr.tile([P, NK, P], BF16, tag="ptr")
            for k in range(NK):
                nc.tensor.matmul(
                    ptr[:, k, :],
                    x_nat[:, k * P : (k + 1) * P],
                    ident,
                    is_transpose=True,
                    start=True,
                    stop=True,
                )
            nc.scalar.copy(xT[:, :, st, :], ptr)

        # --- We prep: load bf16 from the gathered buffer, PE-transpose ---
        for e in range(E):
            for dc in range(NK):  # 8 dout-chunks of 128 rows
                r0 = e * D + dc * P
                we_nat = stage.tile([P, D], BF16, tag="stg")
                nc.sync.dma_start(out=we_nat, in_=we_all_b[r0 : r0 + P, :])
                ptr = psum_tr.tile([P, NK, P], BF16, tag="ptr")
                for k in range(NK):
                    nc.tensor.matmul(
                        ptr[:, k, :],
                        we_nat[:, k * P : (k + 1) * P],
                        ident,
                        is_transpose=True,
                        start=True,
                        stop=True,
                    )
                nc.vector.tensor_copy(weT[:, e, :, dc * P : (dc + 1) * P], ptr)

        # --- gate matmuls: psum += xT.T @ WgT over 8 k-chunks ---
        for st in range(NS):
            pg = psum_gate.tile([P, E], F32, tag="pg")
            for k in range(NK):
                nc.tensor.matmul(
                    pg, xT[:, k, st, :], wgT[:, k, :],
                    start=(k == 0), stop=(k == NK - 1),
                )
            if use_bg:
                nc.vector.tensor_add(gate_all[:, st, :], pg, bg_bc)
            else:
                nc.scalar.copy(gate_all[:, st, :], pg)

        # --- counts -> weights (broadcast to all partitions) ---
        rowmax = const.tile([P, NS], F32)
        nc.vector.tensor_reduce(rowmax, gate_all, axis=mybir.AxisListType.X, op=ALU.max)
        ismax = const.tile([P, E, NS], F32)
        g_ens = gate_all.rearrange("p n e -> p e n")
        rm_bc = bass.AP(
            tensor=rowmax.tensor, offset=rowmax.offset,
            ap=[rowmax.ap[0], [0, E], [1, NS]],
        )
        nc.vector.tensor_tensor(ismax, g_ens, rm_bc, op=ALU.is_ge)
        counts_part = const.tile([P, E], F32)
        nc.vector.tensor_reduce(
            counts_part, ismax, axis=mybir.AxisListType.X, op=ALU.add
        )

        pc1 = psum_gate.tile([1, E], F32, tag="pc1", bufs=1)
        nc.tensor.matmul(pc1, ones_col_f, counts_part, start=True, stop=True)
        counts_sb = const.tile([1, E], F32)
        nc.scalar.copy(counts_sb, pc1)
        pc2 = psum_gate.tile([P, E], F32, tag="pc2", bufs=1)
        nc.tensor.matmul(pc2, ones_row_f, counts_sb, start=True, stop=True)
        w_bc = const.tile([P, E], F32)
        nc.scalar.mul(w_bc, pc2, 1.0 / S)

        psum_gate.release()
        psum_tr.release()

        # --- main expert matmuls + fused epilogue ---
        psum_main = tc.alloc_tile_pool(name="psum_main", bufs=4, space="PSUM")
        relu_p = tc.alloc_tile_pool(name="relu_p", bufs=6)
        acc_p = tc.alloc_tile_pool(name="acc_p", bufs=4)
        out_p = tc.alloc_tile_pool(name="out_p", bufs=3)
        q_p = tc.alloc_tile_pool(name="q_p", bufs=3)
        MAGIC = 8388608.0  # 2^23: x + MAGIC - MAGIC == round-to-nearest-even(x)

        for st in range(NS):
            accs = []
            for half in range(2):
                pts = [
                    psum_main.tile([P, D], F32, tag="pm", name=f"pm{e2}")
                    for e2 in range(2)
                ]
                if use_be:
                    for e2, pt in enumerate(pts):
                        e = half * 2 + e2
                        for c in range(ND):
                            nc.tensor.matmul(
                                pt[:, c * NC : (c + 1) * NC],
                                ones_row_bf,
                                be_bf[e : e + 1, c * NC : (c + 1) * NC],
                                start=True, stop=False,
                            )
                for k in range(NK):
                    lhs = xT[:, k, st, :]
                    for e2, pt in enumerate(pts):
                        for c in range(ND):
                            e = half * 2 + e2
                            nc.tensor.matmul(
                                pt[:, c * NC : (c + 1) * NC],
                                lhs,
                                weT[:, e, k, c * NC : (c + 1) * NC],
                                start=(k == 0 and not use_be),
                                stop=(k == NK - 1),
                            )
                trs = []
                for e2, pt in enumerate(pts):
                    e = half * 2 + e2
                    tr = relu_p.tile([P, D], F32, tag="tr")
                    nc.scalar.activation(tr, pt, RELU, scale=w_bc[:, e : e + 1])
                    trs.append(tr)
                acc = acc_p.tile([P, D], F32, tag="acc")
                nc.vector.tensor_add(acc, trs[0], trs[1])
                accs.append(acc)
            o_f = out_p.tile([P, D], F32, tag="o")
            nc.vector.tensor_add(o_f, accs[0], accs[1])
            # per-token uint8 quantization (y >= 0 post relu-sum)
            rmax = q_p.tile([P, 1], F32, tag="rmax")
            nc.vector.tensor_reduce(rmax, o_f, axis=mybir.AxisListType.X, op=ALU.max)
            qden = q_p.tile([P, 1], F32, tag="qden")
            nc.vector.tensor_scalar(
                qden, rmax, 1e-30, 1.0 / 255.0, op0=ALU.add, op1=ALU.mult
            )
            qs = q_p.tile([P, 1], F32, tag="qs")
            nc.vector.reciprocal(qs, qden)
            q_r = out_p.tile([P, D], F32, tag="qr")
            nc.vector.tensor_scalar(
                q_r, o_f, qs[:, 0:1], MAGIC, op0=ALU.mult, op1=ALU.add
            )
            q_u8 = q_p.tile([P, D], mybir.dt.uint8, tag="q8")
            nc.vector.tensor_scalar(q_u8, q_r, MAGIC, None, op0=ALU.subtract)
            nc.sync.dma_start(out=out_d[st * P : (st + 1) * P, :], in_=q_u8)
            nc.sync.dma_start(out=outs_d[st * P : (st + 1) * P, :], in_=qs)

        q_p.release()
        out_p.release()
        acc_p.release()
        relu_p.release()
        psum_main.release()
        stage.release()
        dram.release()
        big.release()
        const.release()

    _split_sync_waits(nc)
    return nc


_CACHE = {}


def _get_kernel(use_bg: bool, use_be: bool) -> bass.Bass:
    key = (use_bg, use_be)
    if key not in _CACHE:
        _CACHE[key] = build_kernel(use_bg, use_be)
    return _CACHE[key]


def kernel(x, Wg, bg, We, be, _trace=False):
    x = np.asarray(x, dtype=np.float32)
    Wg = np.asarray(Wg, dtype=np.float32)
    bg = np.asarray(bg, dtype=np.float32)
    We = np.asarray(We, dtype=np.float32)
    be = np.asarray(be, dtype=np.float32)
    assert x.shape == (B, S, D) and Wg.shape == (E, D)
    assert We.shape == (E, D, D) and bg.shape == (E,) and be.shape == (E, D)

    use_bg = bool(np.any(bg))
    use_be = bool(np.any(be))
    nc = _get_kernel(use_bg, use_be)

    # host-side bf16 cast + pack: one input tensor per core, We sharded.
    # Threaded: the f32->bf16 casts of x dominate host prep otherwise.
    tail = np.empty((R_TOT - R_WG, D), dtype=NPBF16)
    tail[0:E] = Wg.astype(NPBF16)
    tail[E] = 0
    tail[E, 0:E] = bg.astype(NPBF16)
    tail[E + 1 : E + 1 + E] = be.astype(NPBF16)

    packs = [np.empty((R_TOT, D), dtype=NPBF16) for _ in range(B)]

    def _build_pack(b):
        p = packs[b]
        np.copyto(p[R_X : R_X + S], x[b], casting="unsafe")
        np.copyto(
            p[R_WE : R_WE + WE_SHARD],
            We.reshape(E * D, D)[b * WE_SHARD : (b + 1) * WE_SHARD],
            casting="unsafe",
        )
        p[R_WG:] = tail

    with ThreadPoolExecutor(max_workers=B) as ex:
        list(ex.map(_build_pack, range(B)))

    in_maps = [{"pack": packs[b]} for b in range(B)]
    try:
        res = run_bass_kernel_spmd(
            nc, in_maps, core_ids=list(range(B)), trace=_trace
        )
    except ModuleNotFoundError:
        # NTFF profile hook unavailable in this container; run untraced
        res = run_bass_kernel_spmd(nc, in_maps, core_ids=list(range(B)))
    out = np.empty((B, S, D), dtype=np.float32)

    def _dequant(b):
        q = res.results[b]["out"]
        qs = res.results[b]["out_s"]
        np.multiply(q, np.float32(1.0) / qs, out=out[b])

    with ThreadPoolExecutor(max_workers=B) as ex:
        list(ex.map(_dequant, range(B)))
    if _trace:
        return out, res
    return out


# revision 19
# speedup vs baseline: 4.7029x; 1.1065x over previous
"""MoE routing kernel for Trainium2 (8 NeuronCores, data-parallel over batch).

Problem: x[B=8,S=2048,D=1024] f32; gate Wg[E=4,D]+bg; experts We[E,D,D]+be.
  gate = x @ Wg.T + bg; top1 = argmax(gate); weights[b,e] = count_e(top1[b])/S
  out[b] = sum_e weights[b,e] * relu(x[b] @ We[e].T + be[e])

The warm-path cost in this environment is dominated by host<->device traffic
over the axon tunnel (~38 MB/s), not device compute (~0.3 ms). So the design
minimizes bytes moved:
  - everything is cast to bf16 on the host (x, We, Wg, bg, be) and packed into
    ONE [6153, 1024] bf16 dram tensor per core (one transfer, low latency);
  - the device output is bf16 [S, D], upcast to f32 on the host;
  - measured end-to-end rel err of the full-bf16 pipeline: 5.3e-3 (tolerance
    2e-2; routing argmax from bf16 inputs flips only ~15/16384 tokens).

Sharding: batch dim across the 8 cores (1 batch element per core); expert
weights replicated. No collectives; host gathers per-core outputs.

Per-core kernel structure:
  - PE-transpose x and We tiles so the contraction dim (din) lands on
    partitions; all matmuls in bf16 (PE 1 cyc/row), f32 PSUM accumulation.
  - gate = xT.T @ WgT accumulated over 8 k-chunks in one PSUM tile, then
    argmax->counts->weights entirely on-chip (is_ge + reductions + two tiny
    f32 matmuls for partition-sum and partition-broadcast).
  - expert matmuls: K=1024 contracted in 8 chunks accumulating in PSUM,
    N=512 per matmul (one PSUM bank).
  - epilogue: relu(w_e * y) on ScalarE (w_e >= 0 so the weight folds into the
    activation scale) + DVE add tree, final add emits bf16 out tile.
"""

from concurrent.futures import ThreadPoolExecutor

import numpy as np
import ml_dtypes

import jax

# Persist XLA executables across processes/calls: run_bass_via_pjrt re-jits a
# fresh closure every call, so without this each warm call pays ~0.5s of
# XLA+BIR recompile. Harmless no-op if the PJRT client can't serialize.
jax.config.update("jax_compilation_cache_dir", "/tmp/jax_comp_cache")
jax.config.update("jax_persistent_cache_min_compile_time_secs", 0.0)
jax.config.update("jax_persistent_cache_min_entry_size_bytes", 0)

import concourse.bass as bass
import concourse.tile as tile
from concourse import mybir
from concourse.bass_utils import run_bass_kernel_spmd
from concourse.masks import make_identity
from concourse.vector_clock import ScopedClock, VectorClock

F32 = mybir.dt.float32
BF16 = mybir.dt.bfloat16
RELU = mybir.ActivationFunctionType.Relu
ALU = mybir.AluOpType
NPBF16 = ml_dtypes.bfloat16

B, S, D, E = 8, 2048, 1024, 4
P = 128
NS = S // P   # 16 s-tiles
NK = D // P   # 8 contraction chunks
NC = 512      # matmul moving free dim (one PSUM bank of f32)
ND = D // NC  # 2 dout chunks

# pack row layout (rows of D bf16 values). We is sharded 1/8th per core and
# AllGathered on device — 8MB crosses the slow host tunnel once, not 8 times.
WE_SHARD = E * D // B  # 512 rows per core
R_X = 0                # x[b]:  S rows
R_WE = R_X + S         # We shard: rows [c*512, (c+1)*512) of We.reshape(E*D, D)
R_WG = R_WE + WE_SHARD # Wg:    E rows
R_BG = R_WG + E        # bg:    1 row (first E entries)
R_BE = R_BG + 1        # be:    E rows
R_TOT = R_BE + E       # 2569


def _apply_tile_drain_patch():
    """The walrus build in this container only encodes one sync-wait on a
    CTRL instruction; Tile's kernel-tail drain attaches one wait per active
    proc to a single InstDrain and fails codegen. Split it into one drain
    per proc instead."""
    if getattr(tile.TileContext, "_moe_drain_patch", False):
        return
    tile.TileContext._moe_drain_patch = True

    def _drain_and_barrier(self, tick_clock, wait_clock):
        gc = tick_clock.global_clock
        scopes = [(None, gc)] if isinstance(gc, VectorClock) else gc.items()
        n_emitted = 0
        for scope, vc in scopes:
            n = len(vc)
            for proc in range(n):
                t = vc[proc]
                if t > 0:
                    single = VectorClock([t if i == proc else 0 for i in range(n)])
                    d = self.nc.sync.drain()
                    wait_clock.add_sem_waits(d.ins, ScopedClock({scope: single}))
                    n_emitted += 1
        if n_emitted == 0:
            self.nc.sync.drain()
        self.nc.all_engine_barrier()
        popped = self.nc._tile_sem_poison_stack.pop()
        assert popped is self._sem_poison
        self.nc.clear_and_free_semaphores(list(self.sems.allocated().values()))
        self.nc.all_engine_barrier()

    tile.TileContext._drain_and_barrier = _drain_and_barrier


_apply_tile_drain_patch()


def _split_sync_waits(nc: bass.Bass, limit: int = 1):
    """This container's walrus encodes at most one sync-wait per instruction.
    Hoist excess waits onto same-engine NoOps emitted immediately before the
    instruction — the engine stream blocks on each in turn, which is
    semantically identical to waiting on all of them at once."""
    ctr = 0
    for f in nc.m.functions:
        for bb in f.blocks:
            insts = list(bb.instructions)
            out = []
            changed = False
            for ins in insts:
                si = ins.sync_info
                waits = list(si.on_wait) if si is not None else []
                if len(waits) > limit:
                    changed = True
                    for w in waits[:-limit]:
                        ctr += 1
                        nop = mybir.InstNoOp(name=f"wsplit-{ctr}", ins=[], outs=[])
                        nop.engine = ins.engine
                        nop.sync_info = mybir.SyncInfo(on_wait=[w], on_update=[])
                        out.append(nop)
                    ins.sync_info = mybir.SyncInfo(
                        on_wait=waits[-limit:], on_update=list(si.on_update)
                    )
                out.append(ins)
            if changed:
                bb.instructions = out


def build_kernel(use_bg: bool, use_be: bool) -> bass.Bass:
    nc = bass.Bass()
    pack_d = nc.dram_tensor("pack", [R_TOT, D], BF16, kind="ExternalInput")
    # out is uint8 with a per-token f32 scale: out_f32 ~= q / qs. The device
    # quantizes q = round(y * qs) with qs = 255/(rowmax+eps) and ships qs
    # itself (bitcast into the last 4 uint8 columns of each row), so host
    # dequant by 1/qs cancels any reciprocal error exactly. Halves the d2h
    # bytes vs bf16 at the same global error (<= rowmax/510).
    out_d = nc.dram_tensor("out", [S, D + 4], mybir.dt.uint8, kind="ExternalOutput")

    with tile.TileContext(nc) as tc:
        const = tc.alloc_tile_pool(name="const", bufs=1)
        big = tc.alloc_tile_pool(name="big", bufs=1)
        stage = tc.alloc_tile_pool(name="stage", bufs=4)
        dram = tc.alloc_tile_pool(name="dram", bufs=1, space="DRAM")
        psum_tr = tc.alloc_tile_pool(name="psum_tr", bufs=3, space="PSUM")
        psum_gate = tc.alloc_tile_pool(name="psum_gate", bufs=2, space="PSUM")

        # --- AllGather the We shards into full We (HBM->HBM), first thing so
        # it overlaps the x prep below. Collectives can't touch I/O tensors,
        # so bounce through Internal dram tiles.
        we_in_b = dram.tile([WE_SHARD, D], BF16)
        we_all_b = dram.tile([E * D, D], BF16, addr_space="Shared")
        nc.gpsimd.dma_start(out=we_in_b, in_=pack_d[R_WE : R_WE + WE_SHARD, :])
        nc.gpsimd.collective_compute(
            "AllGather",
            ALU.bypass,
            replica_groups=[list(range(B))],
            ins=[we_in_b.opt()],
            outs=[we_all_b.opt()],
        )

        ident = const.tile([P, P], BF16)
        make_identity(nc, ident)
        ones_col_f = const.tile([P, 1], F32)
        nc.vector.memset(ones_col_f, 1.0)
        ones_row_f = const.tile([1, P], F32)
        nc.vector.memset(ones_row_f, 1.0)

        # --- gate weights: load Wg rows, PE-transpose to [din, e] ---
        wg_sb = const.tile([E, D], BF16)
        nc.sync.dma_start(out=wg_sb, in_=pack_d[R_WG : R_WG + E, :])
        pwg = psum_gate.tile([P, NK, E], BF16, tag="pwg", bufs=1)
        for k in range(NK):
            nc.tensor.matmul(
                pwg[:, k, :],
                wg_sb[0:E, k * P : (k + 1) * P],
                ident[0:E, 0:E],
                is_transpose=True,
                start=True,
                stop=True,
            )
        wgT = const.tile([P, NK, E], BF16)
        nc.scalar.copy(wgT, pwg)

        if use_bg:
            bg_bc = const.tile([P, E], BF16)
            nc.gpsimd.dma_start(
                out=bg_bc,
                in_=bass.AP(tensor=pack_d, offset=R_BG * D, ap=[[0, P], [1, E]]),
            )
        if use_be:
            be_bf = const.tile([E, D], BF16)
            nc.sync.dma_start(out=be_bf, in_=pack_d[R_BE : R_BE + E, :])
            ones_row_bf = const.tile([1, P], BF16)
            nc.vector.memset(ones_row_bf, 1.0)

        # --- persistent transposed operands ---
        xT = big.tile([P, NK, NS, P], BF16)    # 32 KB/partition
        weT = big.tile([P, E, NK, D], BF16)    # 64 KB/partition
        gate_all = const.tile([P, NS, E], F32)

        # --- x prep: load bf16, PE-transpose ---
        for st in range(NS):
            x_nat = stage.tile([P, D], BF16, tag="stg")
            nc.sync.dma_start(out=x_nat, in_=pack_d[st * P : (st + 1) * P, :])
            ptr = psum_tr.tile([P, NK, P], BF16, tag="ptr")
            for k in range(NK):
                nc.tensor.matmul(
                    ptr[:, k, :],
                    x_nat[:, k * P : (k + 1) * P],
                    ident,
                    is_transpose=True,
                    start=True,
                    stop=True,
                )
            nc.scalar.copy(xT[:, :, st, :], ptr)

        # --- We prep: load bf16 from the gathered buffer, PE-transpose ---
        for e in range(E):
            for dc in range(NK):  # 8 dout-chunks of 128 rows
                r0 = e * D + dc * P
                we_nat = stage.tile([P, D], BF16, tag="stg")
                nc.sync.dma_start(out=we_nat, in_=we_all_b[r0 : r0 + P, :])
                ptr = psum_tr.tile([P, NK, P], BF16, tag="ptr")
                for k in range(NK):
                    nc.tensor.matmul(
                        ptr[:, k, :],
                        we_nat[:, k * P : (k + 1) * P],
                        ident,
                        is_transpose=True,
                        start=True,
                        stop=True,
                    )
                nc.vector.tensor_copy(weT[:, e, :, dc * P : (dc + 1) * P], ptr)

        # --- gate matmuls: psum += xT.T @ WgT over 8 k-chunks ---
        for st in range(NS):
            pg = psum_gate.tile([P, E], F32, tag="pg")
            for k in range(NK):
                nc.tensor.matmul(
                    pg, xT[:, k, st, :], wgT[:, k, :],
                    start=(k == 0), stop=(k == NK - 1),
                )
            if use_bg:
                nc.vector.tensor_add(gate_all[:, st, :], pg, bg_bc)
            else:
                nc.scalar.copy(gate_all[:, st, :], pg)

        # --- counts -> weights (broadcast to all partitions) ---
        rowmax = const.tile([P, NS], F32)
        nc.vector.tensor_reduce(rowmax, gate_all, axis=mybir.AxisListType.X, op=ALU.max)
        ismax = const.tile([P, E, NS], F32)
        g_ens = gate_all.rearrange("p n e -> p e n")
        rm_bc = bass.AP(
            tensor=rowmax.tensor, offset=rowmax.offset,
            ap=[rowmax.ap[0], [0, E], [1, NS]],
        )
        nc.vector.tensor_tensor(ismax, g_ens, rm_bc, op=ALU.is_ge)
        counts_part = const.tile([P, E], F32)
        nc.vector.tensor_reduce(
            counts_part, ismax, axis=mybir.AxisListType.X, op=ALU.add
        )

        pc1 = psum_gate.tile([1, E], F32, tag="pc1", bufs=1)
        nc.tensor.matmul(pc1, ones_col_f, counts_part, start=True, stop=True)
        counts_sb = const.tile([1, E], F32)
        nc.scalar.copy(counts_sb, pc1)
        pc2 = psum_gate.tile([P, E], F32, tag="pc2", bufs=1)
        nc.tensor.matmul(pc2, ones_row_f, counts_sb, start=True, stop=True)
        w_bc = const.tile([P, E], F32)
        nc.scalar.mul(w_bc, pc2, 1.0 / S)

        psum_gate.release()
        psum_tr.release()

        # --- main expert matmuls + fused epilogue ---
        psum_main = tc.alloc_tile_pool(name="psum_main", bufs=4, space="PSUM")
        relu_p = tc.alloc_tile_pool(name="relu_p", bufs=6)
        acc_p = tc.alloc_tile_pool(name="acc_p", bufs=4)
        out_p = tc.alloc_tile_pool(name="out_p", bufs=3)
        q_p = tc.alloc_tile_pool(name="q_p", bufs=3)
        MAGIC = 8388608.0  # 2^23: x + MAGIC - MAGIC == round-to-nearest-even(x)

        for st in range(NS):
            accs = []
            for half in range(2):
                pts = [
                    psum_main.tile([P, D], F32, tag="pm", name=f"pm{e2}")
                    for e2 in range(2)
                ]
                if use_be:
                    for e2, pt in enumerate(pts):
                        e = half * 2 + e2
                        for c in range(ND):
                            nc.tensor.matmul(
                                pt[:, c * NC : (c + 1) * NC],
                                ones_row_bf,
                                be_bf[e : e + 1, c * NC : (c + 1) * NC],
                                start=True, stop=False,
                            )
                for k in range(NK):
                    lhs = xT[:, k, st, :]
                    for e2, pt in enumerate(pts):
                        for c in range(ND):
                            e = half * 2 + e2
                            nc.tensor.matmul(
                                pt[:, c * NC : (c + 1) * NC],
                                lhs,
                                weT[:, e, k, c * NC : (c + 1) * NC],
                                start=(k == 0 and not use_be),
                                stop=(k == NK - 1),
                            )
                trs = []
                for e2, pt in enumerate(pts):
                    e = half * 2 + e2
                    tr = relu_p.tile([P, D], F32, tag="tr")
                    nc.scalar.activation(tr, pt, RELU, scale=w_bc[:, e : e + 1])
                    trs.append(tr)
                acc = acc_p.tile([P, D], F32, tag="acc")
                nc.vector.tensor_add(acc, trs[0], trs[1])
                accs.append(acc)
            o_f = out_p.tile([P, D], F32, tag="o")
            nc.vector.tensor_add(o_f, accs[0], accs[1])
            # per-token uint8 quantization (y >= 0 post relu-sum)
            rmax = q_p.tile([P, 1], F32, tag="rmax")
            nc.vector.tensor_reduce(rmax, o_f, axis=mybir.AxisListType.X, op=ALU.max)
            qden = q_p.tile([P, 1], F32, tag="qden")
            nc.vector.tensor_scalar(
                qden, rmax, 1e-30, 1.0 / 255.0, op0=ALU.add, op1=ALU.mult
            )
            qs = q_p.tile([P, 1], F32, tag="qs")
            nc.vector.reciprocal(qs, qden)
            q_r = out_p.tile([P, D], F32, tag="qr")
            nc.vector.tensor_scalar(
                q_r, o_f, qs[:, 0:1], MAGIC, op0=ALU.mult, op1=ALU.add
            )
            q_u8 = q_p.tile([P, D], mybir.dt.uint8, tag="q8")
            nc.vector.tensor_scalar(q_u8, q_r, MAGIC, None, op0=ALU.subtract)
            nc.sync.dma_start(out=out_d[st * P : (st + 1) * P, 0:D], in_=q_u8)
            nc.sync.dma_start(
                out=out_d[st * P : (st + 1) * P, D : D + 4],
                in_=qs.bitcast(mybir.dt.uint8),
            )

        q_p.release()
        out_p.release()
        acc_p.release()
        relu_p.release()
        psum_main.release()
        stage.release()
        dram.release()
        big.release()
        const.release()

    _split_sync_waits(nc)
    return nc


_CACHE = {}


def _get_kernel(use_bg: bool, use_be: bool) -> bass.Bass:
    key = (use_bg, use_be)
    if key not in _CACHE:
        _CACHE[key] = build_kernel(use_bg, use_be)
    return _CACHE[key]


def kernel(x, Wg, bg, We, be, _trace=False):
    x = np.asarray(x, dtype=np.float32)
    Wg = np.asarray(Wg, dtype=np.float32)
    bg = np.asarray(bg, dtype=np.float32)
    We = np.asarray(We, dtype=np.float32)
    be = np.asarray(be, dtype=np.float32)
    assert x.shape == (B, S, D) and Wg.shape == (E, D)
    assert We.shape == (E, D, D) and bg.shape == (E,) and be.shape == (E, D)

    use_bg = bool(np.any(bg))
    use_be = bool(np.any(be))
    nc = _get_kernel(use_bg, use_be)

    # host-side bf16 cast + pack: one input tensor per core, We sharded.
    # Threaded: the f32->bf16 casts of x dominate host prep otherwise.
    tail = np.empty((R_TOT - R_WG, D), dtype=NPBF16)
    tail[0:E] = Wg.astype(NPBF16)
    tail[E] = 0
    tail[E, 0:E] = bg.astype(NPBF16)
    tail[E + 1 : E + 1 + E] = be.astype(NPBF16)

    packs = [np.empty((R_TOT, D), dtype=NPBF16) for _ in range(B)]

    def _build_pack(b):
        p = packs[b]
        np.copyto(p[R_X : R_X + S], x[b], casting="unsafe")
        np.copyto(
            p[R_WE : R_WE + WE_SHARD],
            We.reshape(E * D, D)[b * WE_SHARD : (b + 1) * WE_SHARD],
            casting="unsafe",
        )
        p[R_WG:] = tail

    with ThreadPoolExecutor(max_workers=B) as ex:
        list(ex.map(_build_pack, range(B)))

    in_maps = [{"pack": packs[b]} for b in range(B)]
    try:
        res = run_bass_kernel_spmd(
            nc, in_maps, core_ids=list(range(B)), trace=_trace
        )
    except ModuleNotFoundError:
        # NTFF profile hook unavailable in this container; run untraced
        res = run_bass_kernel_spmd(nc, in_maps, core_ids=list(range(B)))
    out = np.empty((B, S, D), dtype=np.float32)

    def _dequant(b):
        raw = res.results[b]["out"]
        q = raw[:, 0:D]
        qs = np.ascontiguousarray(raw[:, D : D + 4]).view(np.float32)
        np.multiply(q, np.float32(1.0) / qs, out=out[b])

    with ThreadPoolExecutor(max_workers=B) as ex:
        list(ex.map(_dequant, range(B)))
    if _trace:
        return out, res
    return out


# revision 37
# speedup vs baseline: 4.7587x; 1.0119x over previous
"""MoE routing kernel for Trainium2 (8 NeuronCores, data-parallel over batch).

Problem: x[B=8,S=2048,D=1024] f32; gate Wg[E=4,D]+bg; experts We[E,D,D]+be.
  gate = x @ Wg.T + bg; top1 = argmax(gate); weights[b,e] = count_e(top1[b])/S
  out[b] = sum_e weights[b,e] * relu(x[b] @ We[e].T + be[e])

The warm-path cost in this environment is dominated by host<->device traffic
over the axon tunnel (~38 MB/s), not device compute (~0.3 ms). So the design
minimizes bytes moved:
  - everything is cast to bf16 on the host (x, We, Wg, bg, be) and packed into
    ONE [6153, 1024] bf16 dram tensor per core (one transfer, low latency);
  - the device output is bf16 [S, D], upcast to f32 on the host;
  - measured end-to-end rel err of the full-bf16 pipeline: 5.3e-3 (tolerance
    2e-2; routing argmax from bf16 inputs flips only ~15/16384 tokens).

Sharding: batch dim across the 8 cores (1 batch element per core); expert
weights replicated. No collectives; host gathers per-core outputs.

Per-core kernel structure:
  - PE-transpose x and We tiles so the contraction dim (din) lands on
    partitions; all matmuls in bf16 (PE 1 cyc/row), f32 PSUM accumulation.
  - gate = xT.T @ WgT accumulated over 8 k-chunks in one PSUM tile, then
    argmax->counts->weights entirely on-chip (is_ge + reductions + two tiny
    f32 matmuls for partition-sum and partition-broadcast).
  - expert matmuls: K=1024 contracted in 8 chunks accumulating in PSUM,
    N=512 per matmul (one PSUM bank).
  - epilogue: relu(w_e * y) on ScalarE (w_e >= 0 so the weight folds into the
    activation scale) + DVE add tree, final add emits bf16 out tile.
"""

from concurrent.futures import ThreadPoolExecutor

import numpy as np
import ml_dtypes

import jax

# Persist XLA executables across processes/calls: run_bass_via_pjrt re-jits a
# fresh closure every call, so without this each warm call pays ~0.5s of
# XLA+BIR recompile. Harmless no-op if the PJRT client can't serialize.
jax.config.update("jax_compilation_cache_dir", "/tmp/jax_comp_cache")
jax.config.update("jax_persistent_cache_min_compile_time_secs", 0.0)
jax.config.update("jax_persistent_cache_min_entry_size_bytes", 0)

import concourse.bass as bass
import concourse.tile as tile
from concourse import mybir
from concourse.bass_utils import run_bass_kernel_spmd
from concourse.masks import make_identity
from concourse.vector_clock import ScopedClock, VectorClock

F32 = mybir.dt.float32
BF16 = mybir.dt.bfloat16
RELU = mybir.ActivationFunctionType.Relu
ALU = mybir.AluOpType
NPBF16 = ml_dtypes.bfloat16
_PERM = None  # set below after D is defined

B, S, D, E = 8, 2048, 1024, 4
P = 128
MAGIC_F = 8388608.0  # 2^23: x + MAGIC_F - MAGIC_F == round-to-nearest-even(x)
_PERM = np.concatenate([np.arange(0, D, 2), np.arange(1, D, 2)])
NS = S // P   # 16 s-tiles
NK = D // P   # 8 contraction chunks
NC = 512      # matmul moving free dim (one PSUM bank of f32)
ND = D // NC  # 2 dout chunks

# pack row layout (rows of D bf16 values = 2048 bytes). We is sharded 1/8th
# per core and AllGathered on device — 8MB crosses the slow host tunnel once,
# not 8 times. x is int12-quantized (per-core scale): q = round(x/step)+2048
# in [0,4095]; value pairs are nibble-packed into 3 bytes, so a token's 1024
# values occupy 1536 bytes = 768 bf16 slots. The device unpacks and
# dequantizes to bf16 — same accuracy as bf16 x at 75% of the bytes. The din
# axis is stored permuted (evens 0..511, odds 512..1023); We/Wg columns are
# permuted identically on the host so the contraction stays consistent.
WE_SHARD = E * D // B    # 512 rows per core
XROW = 3 * D // 4 // 2   # 384 bf16 slots per 512 values; 768 per token
R_XB = 0                 # x bytes: S*1536 B = 1536 rows
R_WE = R_XB + S * 2 * XROW // D  # 1536
R_WG = R_WE + WE_SHARD   # Wg (din-permuted): E rows
R_BG = R_WG + E          # bg: 1 row (first E entries)
R_BE = R_BG + 1          # be: E rows
R_MISC = R_BE + E        # 1 row: [step_hi, step_lo] bf16
R_TOT = R_MISC + 1       # 2058


def _apply_tile_drain_patch():
    """The walrus build in this container only encodes one sync-wait on a
    CTRL instruction; Tile's kernel-tail drain attaches one wait per active
    proc to a single InstDrain and fails codegen. Split it into one drain
    per proc instead."""
    if getattr(tile.TileContext, "_moe_drain_patch", False):
        return
    tile.TileContext._moe_drain_patch = True

    def _drain_and_barrier(self, tick_clock, wait_clock):
        gc = tick_clock.global_clock
        scopes = [(None, gc)] if isinstance(gc, VectorClock) else gc.items()
        n_emitted = 0
        for scope, vc in scopes:
            n = len(vc)
            for proc in range(n):
                t = vc[proc]
                if t > 0:
                    single = VectorClock([t if i == proc else 0 for i in range(n)])
                    d = self.nc.sync.drain()
                    wait_clock.add_sem_waits(d.ins, ScopedClock({scope: single}))
                    n_emitted += 1
        if n_emitted == 0:
            self.nc.sync.drain()
        self.nc.all_engine_barrier()
        popped = self.nc._tile_sem_poison_stack.pop()
        assert popped is self._sem_poison
        self.nc.clear_and_free_semaphores(list(self.sems.allocated().values()))
        self.nc.all_engine_barrier()

    tile.TileContext._drain_and_barrier = _drain_and_barrier


_apply_tile_drain_patch()


def _split_sync_waits(nc: bass.Bass, limit: int = 1):
    """This container's walrus encodes at most one sync-wait per instruction.
    Hoist excess waits onto same-engine NoOps emitted immediately before the
    instruction — the engine stream blocks on each in turn, which is
    semantically identical to waiting on all of them at once."""
    ctr = 0
    for f in nc.m.functions:
        for bb in f.blocks:
            insts = list(bb.instructions)
            out = []
            changed = False
            for ins in insts:
                si = ins.sync_info
                waits = list(si.on_wait) if si is not None else []
                if len(waits) > limit:
                    changed = True
                    for w in waits[:-limit]:
                        ctr += 1
                        nop = mybir.InstNoOp(name=f"wsplit-{ctr}", ins=[], outs=[])
                        nop.engine = ins.engine
                        nop.sync_info = mybir.SyncInfo(on_wait=[w], on_update=[])
                        out.append(nop)
                    ins.sync_info = mybir.SyncInfo(
                        on_wait=waits[-limit:], on_update=list(si.on_update)
                    )
                out.append(ins)
            if changed:
                bb.instructions = out


def build_kernel(use_bg: bool, use_be: bool) -> bass.Bass:
    nc = bass.Bass()
    # pack is uint16 RAW BITS (bf16 payloads bitcast on load): the x section
    # holds arbitrary packed bytes whose bf16 interpretation includes NaN /
    # denormal patterns — typing it as integer keeps every float-touching
    # layer (sim NaN checks, XLA, the axon transport) from munging them.
    U16 = mybir.dt.uint16
    pack_d = nc.dram_tensor("pack", [R_TOT, D], U16, kind="ExternalInput")
    # out is uint8 with a per-token f32 scale: out_f32 ~= q / qs. The device
    # quantizes q = round(y * qs) with qs = 255/(rowmax+eps) and ships qs
    # itself (bitcast into the last 4 uint8 columns of each row), so host
    # dequant by 1/qs cancels any reciprocal error exactly. Halves the d2h
    # bytes vs bf16 at the same global error (<= rowmax/510).
    out_d = nc.dram_tensor("out", [S, D + 4], mybir.dt.uint8, kind="ExternalOutput")

    with tile.TileContext(nc) as tc:
        const = tc.alloc_tile_pool(name="const", bufs=1)
        big = tc.alloc_tile_pool(name="big", bufs=1)
        stage = tc.alloc_tile_pool(name="stage", bufs=4)
        dram = tc.alloc_tile_pool(name="dram", bufs=1, space="DRAM")
        psum_tr = tc.alloc_tile_pool(name="psum_tr", bufs=3, space="PSUM")
        psum_gate = tc.alloc_tile_pool(name="psum_gate", bufs=2, space="PSUM")

        # --- AllGather the We shards into full We (HBM->HBM), first thing so
        # it overlaps the x prep below. Collectives can't touch I/O tensors,
        # so bounce through Internal dram tiles.
        we_in_b = dram.tile([WE_SHARD, D], U16)
        we_all_b = dram.tile([E * D, D], U16, addr_space="Shared")
        nc.gpsimd.dma_start(out=we_in_b, in_=pack_d[R_WE : R_WE + WE_SHARD, :])
        nc.gpsimd.collective_compute(
            "AllGather",
            ALU.bypass,
            replica_groups=[list(range(B))],
            ins=[we_in_b.opt()],
            outs=[we_all_b.opt()],
        )

        ident = const.tile([P, P], BF16)
        make_identity(nc, ident)
        ones_col_f = const.tile([P, 1], F32)
        nc.vector.memset(ones_col_f, 1.0)
        ones_row_f = const.tile([1, P], F32)
        nc.vector.memset(ones_row_f, 1.0)

        # --- gate weights: load Wg rows, PE-transpose to [din, e] ---
        wg_sb = const.tile([E, D], BF16)
        nc.sync.dma_start(out=wg_sb.bitcast(U16), in_=pack_d[R_WG : R_WG + E, :])
        pwg = psum_gate.tile([P, NK, E], BF16, tag="pwg", bufs=1)
        for k in range(NK):
            nc.tensor.matmul(
                pwg[:, k, :],
                wg_sb[0:E, k * P : (k + 1) * P],
                ident[0:E, 0:E],
                is_transpose=True,
                start=True,
                stop=True,
            )
        wgT = const.tile([P, NK, E], BF16)
        nc.scalar.copy(wgT, pwg)

        if use_bg:
            bg_bc = const.tile([P, E], BF16)
            nc.gpsimd.dma_start(
                out=bg_bc.bitcast(U16),
                in_=bass.AP(tensor=pack_d, offset=R_BG * D, ap=[[0, P], [1, E]]),
            )
        if use_be:
            be_bf = const.tile([E, D], BF16)
            nc.sync.dma_start(
                out=be_bf.bitcast(U16), in_=pack_d[R_BE : R_BE + E, :]
            )
            ones_row_bf = const.tile([1, P], BF16)
            nc.vector.memset(ones_row_bf, 1.0)

        # --- x dequant constants: step = step_hi + step_lo, broadcast [P,1]
        sraw = const.tile([P, 2], BF16)
        nc.gpsimd.dma_start(
            out=sraw.bitcast(U16),
            in_=bass.AP(tensor=pack_d, offset=R_MISC * D, ap=[[0, P], [1, 2]]),
        )
        step_bc = const.tile([P, 1], F32)
        nc.vector.tensor_tensor(step_bc, sraw[:, 0:1], sraw[:, 1:2], op=ALU.add)
        nstep_bc = const.tile([P, 1], F32)
        nc.vector.tensor_scalar(
            nstep_bc, step_bc, -2048.0, None, op0=ALU.mult
        )

        # --- persistent transposed operands ---
        xT = big.tile([P, NK, NS, P], BF16)    # 32 KB/partition
        weT = big.tile([P, E, NK, D], BF16)    # 64 KB/partition
        gate_all = const.tile([P, NS, E], F32)

        # --- x prep: load packed int12 bytes, unpack+dequant to bf16 on DVE,
        # PE-transpose. Per token: 1536 bytes b[3i],b[3i+1],b[3i+2] hold the
        # value pair (q_e, q_o) as (q_e&0xFF, q_o&0xFF, (q_e>>8)|((q_o>>8)<<4)).
        # All arithmetic runs in the DVE f32 ALU on exact small integers;
        # floor(b2/16) uses the +2^23 round trick on (b2/16 - 0.49).
        U8 = mybir.dt.uint8
        HF = D // 2  # 512
        scr = tc.alloc_tile_pool(name="scr", bufs=2)
        for st in range(NS):
            xb = stage.tile([P, 2 * XROW], mybir.dt.uint16, tag="xb")
            nc.sync.dma_start(
                out=xb,
                in_=bass.AP(
                    tensor=pack_d,
                    offset=st * P * 2 * XROW,
                    ap=[[2 * XROW, P], [1, 2 * XROW]],
                ),
            )
            u8 = xb.bitcast(U8)  # [P, 1536]
            bby = [
                bass.AP(tensor=u8.tensor, offset=u8.offset + j, ap=[u8.ap[0], [3, HF]])
                for j in range(3)
            ]
            hp = scr.tile([P, HF], F32, tag="hp")
            nc.vector.tensor_scalar(
                hp, bby[2], 1.0 / 16.0, -0.49, op0=ALU.mult, op1=ALU.add
            )
            hr = scr.tile([P, HF], F32, tag="hr")
            nc.vector.tensor_scalar(hr, hp, MAGIC_F, None, op0=ALU.add)
            h16 = scr.tile([P, HF], F32, tag="h16")
            nc.vector.tensor_scalar(
                h16, hr, MAGIC_F, 16.0, op0=ALU.subtract, op1=ALU.mult
            )
            xq = stage.tile([P, D], BF16, tag="xq")
            # odd values: q_o = b1 + 256*(b2>>4) = b1 + 16*h16
            t16 = scr.tile([P, HF], F32, tag="t16")
            nc.vector.tensor_scalar(t16, h16, 16.0, None, op0=ALU.mult)
            qo = scr.tile([P, HF], F32, tag="qo")
            nc.vector.tensor_tensor(qo, bby[1], t16, op=ALU.add)
            nc.vector.tensor_scalar(
                xq[:, HF:D], qo, step_bc[:, 0:1], nstep_bc[:, 0:1],
                op0=ALU.mult, op1=ALU.add,
            )
            # even values: m = b2 - h16 (= b2 mod 16); q_e = b0 + 256*m
            m = scr.tile([P, HF], F32, tag="m")
            nc.vector.tensor_tensor(m, bby[2], h16, op=ALU.subtract)
            m256 = scr.tile([P, HF], F32, tag="m256")
            nc.vector.tensor_scalar(m256, m, 256.0, None, op0=ALU.mult)
            qe = scr.tile([P, HF], F32, tag="qe")
            nc.vector.tensor_tensor(qe, bby[0], m256, op=ALU.add)
            nc.vector.tensor_scalar(
                xq[:, 0:HF], qe, step_bc[:, 0:1], nstep_bc[:, 0:1],
                op0=ALU.mult, op1=ALU.add,
            )
            ptr = psum_tr.tile([P, NK, P], BF16, tag="ptr")
            for k in range(NK):
                nc.tensor.matmul(
                    ptr[:, k, :],
                    xq[:, k * P : (k + 1) * P],
                    ident,
                    is_transpose=True,
                    start=True,
                    stop=True,
                )
            nc.scalar.copy(xT[:, :, st, :], ptr)
        scr.release()

        # --- We prep: load bf16 from the gathered buffer, PE-transpose ---
        for e in range(E):
            for dc in range(NK):  # 8 dout-chunks of 128 rows
                r0 = e * D + dc * P
                we_nat = stage.tile([P, D], BF16, tag="stg")
                nc.sync.dma_start(
                    out=we_nat.bitcast(U16), in_=we_all_b[r0 : r0 + P, :]
                )
                ptr = psum_tr.tile([P, NK, P], BF16, tag="ptr")
                for k in range(NK):
                    nc.tensor.matmul(
                        ptr[:, k, :],
                        we_nat[:, k * P : (k + 1) * P],
                        ident,
                        is_transpose=True,
                        start=True,
                        stop=True,
                    )
                nc.vector.tensor_copy(weT[:, e, :, dc * P : (dc + 1) * P], ptr)

        # --- gate matmuls: psum += xT.T @ WgT over 8 k-chunks ---
        for st in range(NS):
            pg = psum_gate.tile([P, E], F32, tag="pg")
            for k in range(NK):
                nc.tensor.matmul(
                    pg, xT[:, k, st, :], wgT[:, k, :],
                    start=(k == 0), stop=(k == NK - 1),
                )
            if use_bg:
                nc.vector.tensor_add(gate_all[:, st, :], pg, bg_bc)
            else:
                nc.scalar.copy(gate_all[:, st, :], pg)

        # --- counts -> weights (broadcast to all partitions) ---
        rowmax = const.tile([P, NS], F32)
        nc.vector.tensor_reduce(rowmax, gate_all, axis=mybir.AxisListType.X, op=ALU.max)
        ismax = const.tile([P, E, NS], F32)
        g_ens = gate_all.rearrange("p n e -> p e n")
        rm_bc = bass.AP(
            tensor=rowmax.tensor, offset=rowmax.offset,
            ap=[rowmax.ap[0], [0, E], [1, NS]],
        )
        nc.vector.tensor_tensor(ismax, g_ens, rm_bc, op=ALU.is_ge)
        counts_part = const.tile([P, E], F32)
        nc.vector.tensor_reduce(
            counts_part, ismax, axis=mybir.AxisListType.X, op=ALU.add
        )

        pc1 = psum_gate.tile([1, E], F32, tag="pc1", bufs=1)
        nc.tensor.matmul(pc1, ones_col_f, counts_part, start=True, stop=True)
        counts_sb = const.tile([1, E], F32)
        nc.scalar.copy(counts_sb, pc1)
        pc2 = psum_gate.tile([P, E], F32, tag="pc2", bufs=1)
        nc.tensor.matmul(pc2, ones_row_f, counts_sb, start=True, stop=True)
        w_bc = const.tile([P, E], F32)
        nc.scalar.mul(w_bc, pc2, 1.0 / S)

        psum_gate.release()
        psum_tr.release()

        # --- main expert matmuls + fused epilogue ---
        psum_main = tc.alloc_tile_pool(name="psum_main", bufs=4, space="PSUM")
        relu_p = tc.alloc_tile_pool(name="relu_p", bufs=6)
        acc_p = tc.alloc_tile_pool(name="acc_p", bufs=4)
        out_p = tc.alloc_tile_pool(name="out_p", bufs=3)
        q_p = tc.alloc_tile_pool(name="q_p", bufs=3)
        MAGIC = MAGIC_F

        for st in range(NS):
            accs = []
            for half in range(2):
                pts = [
                    psum_main.tile([P, D], F32, tag="pm", name=f"pm{e2}")
                    for e2 in range(2)
                ]
                if use_be:
                    for e2, pt in enumerate(pts):
                        e = half * 2 + e2
                        for c in range(ND):
                            nc.tensor.matmul(
                                pt[:, c * NC : (c + 1) * NC],
                                ones_row_bf,
                                be_bf[e : e + 1, c * NC : (c + 1) * NC],
                                start=True, stop=False,
                            )
                for k in range(NK):
                    lhs = xT[:, k, st, :]
                    for e2, pt in enumerate(pts):
                        for c in range(ND):
                            e = half * 2 + e2
                            nc.tensor.matmul(
                                pt[:, c * NC : (c + 1) * NC],
                                lhs,
                                weT[:, e, k, c * NC : (c + 1) * NC],
                                start=(k == 0 and not use_be),
                                stop=(k == NK - 1),
                            )
                trs = []
                for e2, pt in enumerate(pts):
                    e = half * 2 + e2
                    tr = relu_p.tile([P, D], F32, tag="tr")
                    nc.scalar.activation(tr, pt, RELU, scale=w_bc[:, e : e + 1])
                    trs.append(tr)
                acc = acc_p.tile([P, D], F32, tag="acc")
                nc.vector.tensor_add(acc, trs[0], trs[1])
                accs.append(acc)
            o_f = out_p.tile([P, D], F32, tag="o")
            nc.vector.tensor_add(o_f, accs[0], accs[1])
            # per-token uint8 quantization (y >= 0 post relu-sum)
            rmax = q_p.tile([P, 1], F32, tag="rmax")
            nc.vector.tensor_reduce(rmax, o_f, axis=mybir.AxisListType.X, op=ALU.max)
            qden = q_p.tile([P, 1], F32, tag="qden")
            nc.vector.tensor_scalar(
                qden, rmax, 1e-30, 1.0 / 255.0, op0=ALU.add, op1=ALU.mult
            )
            qs = q_p.tile([P, 1], F32, tag="qs")
            nc.vector.reciprocal(qs, qden)
            q_r = out_p.tile([P, D], F32, tag="qr")
            nc.vector.tensor_scalar(
                q_r, o_f, qs[:, 0:1], MAGIC, op0=ALU.mult, op1=ALU.add
            )
            q_u8 = q_p.tile([P, D], mybir.dt.uint8, tag="q8")
            nc.vector.tensor_scalar(q_u8, q_r, MAGIC, None, op0=ALU.subtract)
            nc.sync.dma_start(out=out_d[st * P : (st + 1) * P, 0:D], in_=q_u8)
            nc.sync.dma_start(
                out=out_d[st * P : (st + 1) * P, D : D + 4],
                in_=qs.bitcast(mybir.dt.uint8),
            )

        q_p.release()
        out_p.release()
        acc_p.release()
        relu_p.release()
        psum_main.release()
        stage.release()
        dram.release()
        big.release()
        const.release()

    _split_sync_waits(nc)
    return nc


_CACHE = {}


def _get_kernel(use_bg: bool, use_be: bool) -> bass.Bass:
    key = (use_bg, use_be)
    if key not in _CACHE:
        _CACHE[key] = build_kernel(use_bg, use_be)
    return _CACHE[key]


def kernel(x, Wg, bg, We, be, _trace=False):
    x = np.asarray(x, dtype=np.float32)
    Wg = np.asarray(Wg, dtype=np.float32)
    bg = np.asarray(bg, dtype=np.float32)
    We = np.asarray(We, dtype=np.float32)
    be = np.asarray(be, dtype=np.float32)
    assert x.shape == (B, S, D) and Wg.shape == (E, D)
    assert We.shape == (E, D, D) and bg.shape == (E,) and be.shape == (E, D)

    use_bg = bool(np.any(bg))
    use_be = bool(np.any(be))
    nc = _get_kernel(use_bg, use_be)

    # host-side quantize/cast + pack: one input tensor per core, We sharded.
    # din axis permuted evens-then-odds to match the device unpack layout.
    # Threaded: the per-core x int12 packing dominates host prep otherwise.
    We_p = We[:, :, _PERM].reshape(E * D, D)
    tail = np.empty((R_MISC - R_WG, D), dtype=NPBF16)
    tail[0:E] = Wg[:, _PERM].astype(NPBF16)
    tail[E] = 0
    tail[E, 0:E] = bg.astype(NPBF16)
    tail[E + 1 : E + 1 + E] = be.astype(NPBF16)

    # pack dtype is uint16 (raw bits) so packed x bytes never look like bf16
    # NaNs/denormals to any float-handling layer en route to the device.
    packs = [np.empty((R_TOT, D), dtype=np.uint16) for _ in range(B)]

    def _build_pack(b):
        p = packs[b]
        absmax = np.float32(max(np.abs(x[b]).max(), 1e-30))
        step = np.float32(absmax / 2047.5)
        q = np.minimum(
            (x[b] * (np.float32(1.0) / step) + np.float32(2048.5)).astype(
                np.uint16
            ),
            4095,
        )
        qe, qo = q[:, 0::2], q[:, 1::2]
        xb = p[R_XB:R_WE].view(np.uint8).reshape(S, 3 * D // 2)
        xb[:, 0::3] = qe.astype(np.uint8)
        xb[:, 1::3] = qo.astype(np.uint8)
        xb[:, 2::3] = ((qe >> 8) | ((qo >> 8) << 4)).astype(np.uint8)
        p[R_WE : R_WE + WE_SHARD] = (
            We_p[b * WE_SHARD : (b + 1) * WE_SHARD]
            .astype(NPBF16)
            .view(np.uint16)
        )
        p[R_WG:R_MISC] = tail.view(np.uint16)
        p[R_MISC] = 0
        srow = np.zeros(2, dtype=NPBF16)
        srow[0] = step
        srow[1] = np.float32(step) - np.float32(srow[0])
        p[R_MISC, 0:2] = srow.view(np.uint16)

    with ThreadPoolExecutor(max_workers=B) as ex:
        list(ex.map(_build_pack, range(B)))

    in_maps = [{"pack": packs[b]} for b in range(B)]
    try:
        res = run_bass_kernel_spmd(
            nc, in_maps, core_ids=list(range(B)), trace=_trace
        )
    except ModuleNotFoundError:
        # NTFF profile hook unavailable in this container; run untraced
        res = run_bass_kernel_spmd(nc, in_maps, core_ids=list(range(B)))
    out = np.empty((B, S, D), dtype=np.float32)

    def _dequant(b):
        raw = res.results[b]["out"]
        q = raw[:, 0:D]
        qs = np.ascontiguousarray(raw[:, D : D + 4]).view(np.float32)
        np.multiply(q, np.float32(1.0) / qs, out=out[b])

    with ThreadPoolExecutor(max_workers=B) as ex:
        list(ex.map(_dequant, range(B)))
    if _trace:
        return out, res
    return out
